# revision 2
# baseline (speedup 1.0000x reference)
"""BLT local encoder (2-layer transformer, patch-equality block-diagonal attention)
on 8 Trainium2 NeuronCores — v3.

Sharding: each of the 4 sequences is split at a patch boundary near S/2 into 2
independent shards -> 8 shards, one per core, no cross-core communication.

Structure:
- tok_emb gather on host (baseT = tok+pos+hash).
- LN gains folded into following weights on host; LN biases folded into
  projection biases; k-bias dropped (softmax-invariant); v-bias folded into
  the output-projection bias.  Kernel LN = (x-mu)*rstd only.
- K/Q/V/W1 projections run as error-compensated fp8 DoubleRow matmuls:
  weights W ~= (W8 + R8)/64 and activations x ~= x8 + r8, all fp8e4m3 at a
  shared scale, so the three products W8x8 + W8r8 + R8x8 accumulate in one
  PSUM group (the dropped R8r8 term is ~0.1%%*3.6%%).  DoubleRow contracts
  256 rows per matmul.  Precision matches bf16 (~2e-3 end to end).
- O-projection / W2 / attention-score path stay bf16.
- Attention: narrow windows — key tile j only sees queries within AW=32 of
  the tile (max patch run here is 11; host asserts run <= AW+1).  Softmax
  denominator merged into the ctx matmul via a ones-plane in Vext.
- QKVO loaded once per layer; W1/W2 streamed once per 384-token chunk.
- ctx overwrites QT per head (partition-disjoint); uT reuses KT's pool slot
  (KT is dead after the score matmuls).
"""

import numpy as np
import ml_dtypes

import concourse.bass as bass
import concourse.tile as tile
from concourse import bacc, bass_utils, mybir

F32 = mybir.dt.float32
F32R = mybir.dt.float32r
BF16 = mybir.dt.bfloat16
FP8 = mybir.dt.float8e4
AF = mybir.ActivationFunctionType
OP = mybir.AluOpType
DR = mybir.MatmulPerfMode.DoubleRow

B, S, D, H, F, L = 4, 2048, 1024, 16, 4096, 2
DH = D // H  # 64
DC = D // 128  # 8
FC = F // 128  # 32
EPS = 1e-5
SCALE = 1.0 / np.sqrt(DH)

P = 128
NT = 9           # token tiles per shard
PT = NT * P      # 1152
TC = 384         # token chunk
NCH = 3
NCORES = 8
BF = ml_dtypes.bfloat16
E4 = ml_dtypes.float8_e4m3
AW = 32          # attention window margin (max patch run must be <= AW+1)
WW = 128 + 2 * AW  # 192: per-key-tile query window width
WS = 64.0        # fp8 weight scale
IWS = 1.0 / WS


def _build():
    nc = bacc.Bacc("TRN2", target_bir_lowering=False, debug=False,
                   num_devices=NCORES)

    def din(name, shape, dt=F32):
        return nc.dram_tensor(name, shape, dt, kind="ExternalInput").ap()

    baseT = din("baseT", [P, DC * PT], F32R)   # LN0-applied residual (host)
    x80_d = din("x80", [P, DC * PT], FP8)      # layer-0 LN1 output (host)
    xr0_d = din("xr0", [P, DC * PT], FP8)      # its fp8 residual (host)
    masks_d = din("masks", [P, NT * 2 * WW], BF16)
    wq8, wqr, wk8, wkr, wv8, wvr = [], [], [], [], [], []
    w18, w1r, wo, w2 = [], [], [], []
    bq, bo, b1, b2 = [], [], [], []
    for l in range(L):
        wq8.append(din(f"wq8_{l}", [P, DC * D], FP8))
        wqr.append(din(f"wqr_{l}", [P, DC * D], FP8))
        wk8.append(din(f"wk8_{l}", [P, DC * D], FP8))
        wkr.append(din(f"wkr_{l}", [P, DC * D], FP8))
        wv8.append(din(f"wv8_{l}", [P, DC * D], FP8))
        wvr.append(din(f"wvr_{l}", [P, DC * D], FP8))
        w18.append(din(f"w18_{l}", [P, DC * F], FP8))
        w1r.append(din(f"w1r_{l}", [P, DC * F], FP8))
        wo.append(din(f"wo{l}", [P, DC * D], BF16))
        w2.append((din(f"w28_{l}", [P, FC * D], FP8),
                   din(f"w2r_{l}", [P, FC * D], FP8)))
        bq.append(din(f"bq{l}", [D]))
        bo.append(din(f"bo{l}", [D]))
        b1.append(din(f"b1{l}", [F]))
        b2.append(din(f"b2{l}", [D]))
    b2r64 = din("b2r64", [1, L * D], BF16)   # 64*b2 row-major per layer
    houtT = nc.dram_tensor("houtT", [P, DC * PT], F32R, kind="ExternalOutput").ap()

    with tile.TileContext(nc) as tc:
        with (
            nc.allow_low_precision(reason="fp8/bf16 compensated path within budget"),
            tc.tile_pool(name="pers", bufs=1) as pers,
            tc.tile_pool(name="x8p", bufs=1) as x8p,
            tc.tile_pool(name="kup", bufs=1) as kup,
            tc.tile_pool(name="qtp", bufs=1) as qtp,
            tc.tile_pool(name="vxp", bufs=1) as vxp,
            tc.tile_pool(name="estp", bufs=2) as estp,
            tc.tile_pool(name="wp8", bufs=6) as wp8,
            tc.tile_pool(name="wpb", bufs=3) as wpb,
            tc.tile_pool(name="lnp", bufs=4) as lnp,
            tc.tile_pool(name="rmp", bufs=2) as rmp,
            tc.tile_pool(name="small", bufs=2) as small,
            tc.tile_pool(name="nrmp", bufs=2) as nrmp,
            tc.tile_pool(name="pp", bufs=8, space="PSUM") as pp,
        ):
            # Tiles allocated up front; the DMAs that fill cpack / masks /
            # hT are emitted late (inside layer 0) so the DMA queue serves
            # layer-0's critical path (x80/xr0/wk) first.
            hT = pers.tile([P, DC * PT], F32R, tag="hT")
            hT3 = hT.rearrange("p (c t) -> p c t", c=DC)
            cpack = pers.tile([P, 132], F32, tag="cpack")
            nc.vector.memset(cpack[:, 0:1], 1.0)
            nc.vector.memset(cpack[0:1, 1:2], EPS)
            bcol = [cpack[:, 18 + 56 * l:18 + 56 * l + 56] for l in range(L)]
            eps_t = cpack[0:1, 1:2]
            ones_col = pers.tile([P, 1], F32R, tag="ones_col")
            nc.vector.tensor_copy(ones_col, cpack[:, 0:1])
            ones_bf = pers.tile([P, 1], BF16, tag="ones_bf")
            nc.vector.tensor_copy(ones_bf, cpack[:, 0:1])
            mk = pers.tile([P, NT * 2 * WW], BF16, tag="mk")
            ones_row = pers.tile([1, TC], BF16, tag="ones_row")
            nc.vector.memset(ones_row, 1.0)
            b2row = pers.tile([1, L * D], BF16, tag="b2row")

            def emit_late_dmas():
                nc.sync.dma_start(out=b2row, in_=b2r64)
                for l in range(L):
                    b0 = 18 + 56 * l
                    for i, v in enumerate((bq[l], bo[l], b2[l])):
                        nc.sync.dma_start(
                            out=cpack[:, b0 + 8 * i:b0 + 8 * i + 8],
                            in_=v.rearrange("(c p) -> p c", p=P))
                    nc.sync.dma_start(out=cpack[:, b0 + 24:b0 + 56],
                                      in_=b1[l].rearrange("(c p) -> p c", p=P))
                nc.sync.dma_start(out=mk, in_=masks_d)
                baseT3 = baseT.rearrange("p (c t) -> p c t", c=DC)
                for ci in range(NCH):
                    t0 = ci * TC
                    nc.sync.dma_start(out=hT3[:, :, t0:t0 + TC],
                                      in_=baseT3[:, :, t0:t0 + TC])

            def ln_chunk(ci, out_pair=None, gb=None):
                """LayerNorm over features (partitions) for token chunk ci.
                gb=(g,b): write (x-mu)*rstd*g+b back into hT (float32).
                out_pair=(x8,xr): write fp8 value + fp8 residual."""
                t0 = ci * TC
                ps1 = pp.tile([1, TC], F32, tag="mm", name=f"lns1_{ci}")
                ps2 = pp.tile([1, TC], F32, tag="mm", name=f"lns2_{ci}")
                for dc in range(DC):
                    hsl = hT[:, dc * PT + t0:dc * PT + t0 + TC]
                    nc.tensor.matmul(ps1, lhsT=ones_col, rhs=hsl,
                                     start=(dc == 0), stop=(dc == DC - 1))
                    sq = lnp.tile([P, TC], BF16, tag="lnsq", name=f"lnsq{dc}")
                    nc.scalar.square(sq, hsl)
                    nc.tensor.matmul(ps2, lhsT=ones_bf, rhs=sq,
                                     start=(dc == 0), stop=(dc == DC - 1))
                mean = small.tile([1, TC], F32, tag="smf", name="mean")
                var = small.tile([1, TC], F32, tag="smf", name="var")
                rstd = small.tile([1, TC], BF16, tag="smb", name="rstd")
                mr = small.tile([1, TC], BF16, tag="smb", name="mr")
                nc.vector.tensor_scalar_mul(mean, ps1, 1.0 / D)
                nc.vector.tensor_mul(var, mean, mean)
                nc.vector.scalar_tensor_tensor(var, ps2, 1.0 / D, var,
                                               op0=OP.mult, op1=OP.subtract)
                nc.scalar.activation(var, var, AF.Sqrt, bias=eps_t)
                nc.vector.reciprocal(rstd, var)
                nc.vector.tensor_mul(mr, mean, rstd)
                RM = rmp.tile([P, 2 * TC], BF16, tag="lnRM")
                nc.gpsimd.partition_broadcast(RM[:, 0:TC], rstd)
                nc.gpsimd.partition_broadcast(RM[:, TC:2 * TC], mr)
                for dc in range(DC):
                    hsl = hT[:, dc * PT + t0:dc * PT + t0 + TC]
                    d1 = lnp.tile([P, TC], F32, tag="lnt", name=f"lnd{dc}")
                    nc.vector.tensor_mul(d1, hsl, RM[:, 0:TC])
                    if gb is not None:
                        d2 = lnp.tile([P, TC], F32, tag="lnt", name=f"lnd2_{dc}")
                        nc.vector.tensor_sub(d2, d1, RM[:, TC:2 * TC])
                        gcol, bc = gb
                        nc.vector.tensor_scalar(
                            hsl, d2, gcol[:, dc:dc + 1], bc[:, dc:dc + 1],
                            op0=OP.mult, op1=OP.add)
                    else:
                        t = lnp.tile([P, TC], F32, tag="lnt", name=f"lnx{dc}")
                        nc.vector.tensor_sub(t, d1, RM[:, TC:2 * TC])
                        x8sl = out_pair[0][:, dc * PT + t0:dc * PT + t0 + TC]
                        r8sl = out_pair[1][:, dc * PT + t0:dc * PT + t0 + TC]
                        nc.scalar.activation(x8sl, t, AF.Copy)
                        nc.vector.tensor_sub(r8sl, t, x8sl)



            def dr_accum(ps, w8v, wrv, xa3, xr3a, c0, cw, t0, tw):
                """12 DoubleRow MMs: psum += (W8+R8)^T(x8+r8) minus R8r8,
                contraction over all 8 dc blocks (4 pairs x 3 terms)."""
                n = 0
                for dcp in range(4):
                    hf, lc = dcp // 2, (2 * dcp) % 4
                    w8sl = w8v[hf][:, lc:lc + 2, c0:c0 + cw]
                    wrsl = wrv[hf][:, lc:lc + 2, c0:c0 + cw]
                    x8sl = xa3[:, 2 * dcp:2 * dcp + 2, t0:t0 + tw]
                    xrsl = xr3a[:, 2 * dcp:2 * dcp + 2, t0:t0 + tw]
                    for lh, rh in ((w8sl, x8sl), (w8sl, xrsl), (wrsl, x8sl)):
                        nc.tensor.matmul(ps, lhsT=lh, rhs=rh,
                                         start=(n == 0), stop=(n == 11),
                                         perf_mode=DR)
                        n += 1

            def dr_accum_tok(ps, w8v, wrv, xa3, xr3a, c0, cw, tt):
                """Token-major variant: lhsT = activation pairs."""
                n = 0
                for dcp in range(4):
                    hf, lc = dcp // 2, (2 * dcp) % 4
                    x8sl = xa3[:, 2 * dcp:2 * dcp + 2, tt * 128:tt * 128 + 128]
                    xrsl = xr3a[:, 2 * dcp:2 * dcp + 2, tt * 128:tt * 128 + 128]
                    w8sl = w8v[hf][:, lc:lc + 2, c0:c0 + cw]
                    wrsl = wrv[hf][:, lc:lc + 2, c0:c0 + cw]
                    for lh, rh in ((x8sl, w8sl), (xrsl, w8sl), (x8sl, wrsl)):
                        nc.tensor.matmul(ps, lhsT=lh, rhs=rh,
                                         start=(n == 0), stop=(n == 11),
                                         perf_mode=DR)
                        n += 1

            def load_w8(dram8, dramr, tag, l):
                """Load an fp8 [D,D]-class matrix + residual as 2 half tiles
                each, returning ([P,4,1024] views x 2 halves) for both."""
                v8, vr = [], []
                for hf in range(2):
                    t8 = wp8.tile([P, 4 * D], FP8, tag="w8", name=f"{tag}8_{l}_{hf}")
                    nc.sync.dma_start(out=t8, in_=dram8[:, hf * 4 * D:(hf + 1) * 4 * D])
                    v8.append(t8.rearrange("p (c d) -> p c d", c=4))
                    tr = wp8.tile([P, 4 * D], FP8, tag="w8", name=f"{tag}r_{l}_{hf}")
                    nc.sync.dma_start(out=tr, in_=dramr[:, hf * 4 * D:(hf + 1) * 4 * D])
                    vr.append(tr.rearrange("p (c d) -> p c d", c=4))
                return v8, vr

            # ---------- layers ----------
            for l in range(L):
                # ---- LN1 -> x8 + residual (fp8, full shard) ----
                # layer 0's LN1 is input-only: host ships it pre-computed
                x8 = x8p.tile([P, DC * PT], FP8, tag="x8", name=f"x8_{l}")
                xr = x8p.tile([P, DC * PT], FP8, tag="xr", name=f"xr_{l}")
                if l == 0:
                    x8c = x8.rearrange("p (c t) -> p c t", c=DC)
                    xrc = xr.rearrange("p (c t) -> p c t", c=DC)
                    x80c = x80_d.rearrange("p (c t) -> p c t", c=DC)
                    xr0c = xr0_d.rearrange("p (c t) -> p c t", c=DC)
                    for ci in range(NCH):
                        t0 = ci * TC
                        nc.sync.dma_start(out=x8c[:, :, t0:t0 + TC],
                                          in_=x80c[:, :, t0:t0 + TC])
                        nc.sync.dma_start(out=xrc[:, :, t0:t0 + TC],
                                          in_=xr0c[:, :, t0:t0 + TC])
                else:
                    for ci in range(NCH):
                        ln_chunk(ci, out_pair=(x8, xr))
                x83 = x8.rearrange("p (c t) -> p c t", c=DC)
                xr3 = xr.rearrange("p (c t) -> p c t", c=DC)

                # ---- K (feature-major KT) and V (token-major Vext) ----
                KT = kup.tile([P, DC * PT], BF16, tag="ku", name=f"KT{l}")
                Vx = vxp.tile([P, 65, NT * H], BF16, tag="vx", name=f"Vx{l}")
                nc.vector.memset(Vx[:, 64:65, :], 1.0)

                wkb8, wkbr = load_w8(wk8[l], wkr[l], "wk", l)
                for ci in range(NCH):
                    t0 = ci * TC
                    for og in range(2):
                        pss = [pp.tile([P, TC], F32, tag="mm", name=f"psk{i}")
                               for i in range(4)]
                        for o4 in range(4):
                            oc = og * 4 + o4
                            dr_accum(pss[o4], wkb8, wkbr, x83, xr3,
                                     oc * 128, 128, t0, TC)
                        for o4 in range(4):
                            oc = og * 4 + o4
                            nc.scalar.activation(
                                KT[:, oc * PT + t0:oc * PT + t0 + TC], pss[o4],
                                AF.Copy, scale=IWS)

                wvb8, wvbr = load_w8(wv8[l], wvr[l], "wv", l)
                for tt in range(NT):
                    for nh in range(2):
                        psv = pp.tile([P, 512], F32, tag="mm", name=f"psv{tt}_{nh}")
                        dr_accum_tok(psv, wvb8, wvbr, x83, xr3,
                                     nh * 512, 512, tt)
                        nc.scalar.activation(
                            Vx[:, 0:64, tt * H + nh * 8:tt * H + nh * 8 + 8],
                            psv.rearrange("p (h x) -> p x h", h=8),
                            AF.Copy, scale=IWS)

                # ---- Q (feature-major QT, bias bq') ----
                QT = qtp.tile([P, DC * PT], BF16, tag="qt", name=f"QT{l}")
                wqb8, wqbr = load_w8(wq8[l], wqr[l], "wq", l)
                if l == 0:
                    emit_late_dmas()
                for ci in range(NCH):
                    t0 = ci * TC
                    for og in range(2):
                        psq = [pp.tile([P, TC], F32, tag="mm", name=f"psq{i}")
                               for i in range(4)]
                        for o4 in range(4):
                            oc = og * 4 + o4
                            dr_accum(psq[o4], wqb8, wqbr, x83, xr3,
                                     oc * 128, 128, t0, TC)
                        for o4 in range(4):
                            oc = og * 4 + o4
                            nc.scalar.activation(
                                QT[:, oc * PT + t0:oc * PT + t0 + TC], psq[o4],
                                AF.Identity, bias=bcol[l][:, oc:oc + 1], scale=IWS)

                # ---- attention; ctx overwrites QT regions per head ----
                ests = {}

                def w0_of(j):
                    return min(max(j * 128 - AW, 0), PT - WW)

                def attn_scores(i):
                    est = estp.tile([P, NT * 384], BF16, tag="est", name=f"est{i}")
                    ests[i] = est
                    for j in range(NT):
                        w0 = w0_of(j)
                        for ho in range(2):
                            po = ho * 64
                            pst = pp.tile([P, 384], F32, tag="mm", name=f"pst{j}_{ho}")
                            nc.tensor.matmul(
                                pst[:, 0:WW],
                                lhsT=KT[po:po + 64, i * PT + j * 128:i * PT + j * 128 + 128],
                                rhs=QT[po:po + 64, i * PT + w0:i * PT + w0 + WW],
                                start=True, stop=True)
                            esl = est[:, j * 384 + ho * WW:j * 384 + (ho + 1) * WW]
                            nc.scalar.activation(esl, pst[:, 0:WW], AF.Exp,
                                                 scale=float(SCALE))
                        ej = est[:, j * 384:j * 384 + 2 * WW]
                        nc.vector.tensor_mul(ej, ej, mk[:, j * 2 * WW:(j + 1) * 2 * WW])

                def attn_ctx(i):
                    est = ests.pop(i)
                    for ho in range(2):
                        h = 2 * i + ho
                        po = ho * 64
                        for c in range(NCH):
                            psc = pp.tile([P, TC], F32, tag="mm", name=f"psc{h}_{c}")
                            mms = []
                            for qi in range(3):
                                qt = 3 * c + qi
                                w0c = w0_of(qt)
                                mms.append((qi * 128, 128, qt, qt * 128 - w0c, True))
                                if qt - 1 >= 0:
                                    w0e = w0_of(qt - 1)
                                    mms.append((qi * 128, AW, qt - 1,
                                                qt * 128 - w0e, False))
                                if qt + 1 < NT:
                                    w0e = w0_of(qt + 1)
                                    mms.append((qi * 128 + 128 - AW, AW, qt + 1,
                                                (qt + 1) * 128 - AW - w0e, False))
                            for mi, (pcol, wdt, j, ecol, st) in enumerate(mms):
                                nc.tensor.matmul(
                                    psc[0:65, pcol:pcol + wdt],
                                    lhsT=Vx[:, 0:65, j * H + h:j * H + h + 1],
                                    rhs=est[:, j * 384 + ho * WW + ecol:
                                            j * 384 + ho * WW + ecol + wdt],
                                    start=st, stop=(mi == len(mms) - 1),
                                    skip_group_check=True)
                            den = nrmp.tile([1, TC], BF16, tag="den", name=f"den{h}_{c}")
                            rnm = nrmp.tile([64, TC], BF16, tag="rnm", name=f"rnm{h}_{c}")
                            nc.vector.reciprocal(den, psc[64:65, :])
                            nc.gpsimd.partition_broadcast(rnm, den)
                            nc.vector.tensor_mul(
                                QT[po:po + 64, i * PT + c * TC:i * PT + c * TC + TC],
                                psc[0:64, :], rnm)

                attn_scores(0)
                for i in range(1, H // 2):
                    attn_scores(i)
                    attn_ctx(i - 1)
                attn_ctx(H // 2 - 1)

                # ---- O-projection + residual (ctx lives in QT; bf16) ----
                wob = []
                for hf in range(2):
                    t = wpb.tile([P, 4 * D], BF16, tag="w", name=f"wob{l}_{hf}")
                    nc.sync.dma_start(out=t, in_=wo[l][:, hf * 4 * D:(hf + 1) * 4 * D])
                    wob.append(t)
                for ci in range(NCH):
                    t0 = ci * TC
                    for og in range(2):
                        pso = [pp.tile([P, TC], F32, tag="mm", name=f"pso{i}")
                               for i in range(4)]
                        for di in range(DC):
                            wb = wob[di // 4]
                            for o4 in range(4):
                                do_ = og * 4 + o4
                                nc.tensor.matmul(
                                    pso[o4],
                                    lhsT=wb[:, (di % 4) * D + do_ * 128:(di % 4) * D + do_ * 128 + 128],
                                    rhs=QT[:, di * PT + t0:di * PT + t0 + TC],
                                    start=(di == 0), stop=(di == DC - 1))
                        for o4 in range(4):
                            do_ = og * 4 + o4
                            hsl = hT[:, do_ * PT + t0:do_ * PT + t0 + TC]
                            nc.vector.scalar_tensor_tensor(
                                hsl, pso[o4], bcol[l][:, 8 + do_:8 + do_ + 1], hsl,
                                op0=OP.add, op1=OP.add)

                # ---- LN2 -> x8/xr (reuses the x8p slots) ----
                x28 = x8p.tile([P, DC * PT], FP8, tag="x8", name=f"x28_{l}")
                x2r = x8p.tile([P, DC * PT], FP8, tag="xr", name=f"x2r_{l}")
                for ci in range(NCH):
                    ln_chunk(ci, out_pair=(x28, x2r))
                x283 = x28.rearrange("p (c t) -> p c t", c=DC)
                x2r3 = x2r.rearrange("p (c t) -> p c t", c=DC)

                # ---- FFN: W1 and W2 both fp8-DR, compensated uT ----
                for ci in range(NCH):
                    t0 = ci * TC
                    # u8 in first half, its fp8 residual ur in second half
                    uT = kup.tile([P, 2 * FC * TC], FP8, tag="ku", name=f"uT{l}_{ci}")
                    UR = FC * TC
                    for fg in range(4):
                        p8, pr = [], []
                        for hf in range(2):
                            t8 = wp8.tile([P, 4096], FP8, tag="w8",
                                          name=f"w18p{l}_{ci}_{fg}_{hf}")
                            nc.sync.dma_start(
                                out=t8,
                                in_=w18[l][:, fg * 8192 + hf * 4096:fg * 8192 + (hf + 1) * 4096])
                            p8.append(t8.rearrange("p (c d) -> p c d", c=4))
                            tr = wp8.tile([P, 4096], FP8, tag="w8",
                                          name=f"w1rp{l}_{ci}_{fg}_{hf}")
                            nc.sync.dma_start(
                                out=tr,
                                in_=w1r[l][:, fg * 8192 + hf * 4096:fg * 8192 + (hf + 1) * 4096])
                            pr.append(tr.rearrange("p (c d) -> p c d", c=4))
                        for fgi in range(2):
                            psf = [pp.tile([P, TC], F32, tag="mm", name=f"psf{i}")
                                   for i in range(4)]
                            for f4 in range(4):
                                fcl = fgi * 4 + f4
                                dr_accum(psf[f4], p8, pr, x283, x2r3,
                                         fcl * 128, 128, t0, TC)
                            for f4 in range(4):
                                fc = fg * 8 + fgi * 4 + f4
                                gt = lnp.tile([P, TC], F32, tag="lnt",
                                              name=f"gt{fc}")
                                nc.scalar.activation(
                                    gt, psf[f4], AF.Gelu,
                                    bias=bcol[l][:, 24 + fc:24 + fc + 1], scale=IWS)
                                u8sl = uT[:, fc * TC:(fc + 1) * TC]
                                ursl = uT[:, UR + fc * TC:UR + (fc + 1) * TC]
                                if fc % 2 == 0:
                                    nc.scalar.activation(u8sl, gt, AF.Copy)
                                else:
                                    nc.vector.tensor_copy(u8sl, gt)
                                nc.vector.tensor_sub(ursl, gt, u8sl)
                    def w2_piece(psh, nmm, dg, pc):
                        wb8 = wp8.tile([P, 4096], FP8, tag="w8",
                                       name=f"w28p{l}_{ci}_{dg}_{pc}")
                        nc.sync.dma_start(
                            out=wb8,
                            in_=w2[l][0][:, dg * 16384 + pc * 4096:dg * 16384 + (pc + 1) * 4096])
                        wbr = wp8.tile([P, 4096], FP8, tag="w8",
                                       name=f"w2rp{l}_{ci}_{dg}_{pc}")
                        nc.sync.dma_start(
                            out=wbr,
                            in_=w2[l][1][:, dg * 16384 + pc * 4096:dg * 16384 + (pc + 1) * 4096])
                        w83 = wb8.rearrange("p (c d) -> p c d", c=8)
                        wr3 = wbr.rearrange("p (c d) -> p c d", c=8)
                        u83 = uT.rearrange("p (c t) -> p c t", c=2 * FC)
                        for fp_ in range(4):
                            fc = pc * 8 + 2 * fp_
                            u8sl = u83[:, fc:fc + 2, :]
                            ursl = u83[:, FC + fc:FC + fc + 2, :]
                            for o4 in range(4):
                                w8sl = w83[:, 2 * fp_:2 * fp_ + 2,
                                           o4 * 128:o4 * 128 + 128]
                                wrsl = wr3[:, 2 * fp_:2 * fp_ + 2,
                                           o4 * 128:o4 * 128 + 128]
                                for lh, rh in ((w8sl, u8sl), (w8sl, ursl),
                                               (wrsl, u8sl)):
                                    nc.tensor.matmul(
                                        psh[o4], lhsT=lh, rhs=rh,
                                        start=(nmm[o4] == 0), stop=False,
                                        perf_mode=DR)
                                    nmm[o4] += 1

                    for dg in range(2):
                        psh = [pp.tile([P, TC], F32, tag="mm", name=f"psh{i}")
                               for i in range(4)]
                        nmm = [0] * 4
                        for pc in range(4):
                            w2_piece(psh, nmm, dg, pc)
                        for o4 in range(4):
                            do_ = dg * 4 + o4
                            # + 64*b2 via a rank-1 matmul, closing the group
                            nc.tensor.matmul(
                                psh[o4],
                                lhsT=b2row[:, l * D + do_ * 128:l * D + do_ * 128 + 128],
                                rhs=ones_row,
                                start=False, stop=True)
                        for o4 in range(4):
                            do_ = dg * 4 + o4
                            hsl = hT[:, do_ * PT + t0:do_ * PT + t0 + TC]
                            nc.vector.scalar_tensor_tensor(
                                hsl, psh[o4], IWS, hsl,
                                op0=OP.mult, op1=OP.add)

            # chunked output DMA: chunk c leaves as soon as its last residual
            # add lands
            houtT3 = houtT.rearrange("p (c t) -> p c t", c=DC)
            for ci in range(NCH):
                t0 = ci * TC
                nc.sync.dma_start(out=houtT3[:, :, t0:t0 + TC],
                                  in_=hT3[:, :, t0:t0 + TC])

    nc.compile()
    return nc


_NC_CACHE = {}


def _get_nc():
    if "nc" not in _NC_CACHE:
        _NC_CACHE["nc"] = _build()
    return _NC_CACHE["nc"]


def _ln_np(x, g=None, bta=None):
    mu = x.mean(-1, keepdims=True)
    var = ((x - mu) ** 2).mean(-1, keepdims=True)
    y = (x - mu) / np.sqrt(var + EPS)
    if g is not None:
        y = y * g + bta
    return y.astype(np.float32)


def _prep_core(inputs, tokemb_f32, ln0g, ln0b, b, start, n):
    """Per-core in_map entries that depend on the shard."""
    ids = np.asarray(inputs["input_ids"][b, start:start + n]).astype(np.int64)
    pid = np.asarray(inputs["patch_ids"][b, start:start + n]).astype(np.int64)
    pos_emb = np.asarray(inputs["pos_emb"], np.float32)
    hashes = np.asarray(inputs["hash_embeddings"], np.float32)

    base = np.zeros((PT, D), np.float32)
    emb = tokemb_f32[ids] + pos_emb[start:start + n] + hashes[b, start:start + n]
    base[:n] = _ln_np(emb, ln0g, ln0b)
    baseT = np.ascontiguousarray(
        base.reshape(PT, DC, P).transpose(2, 1, 0).reshape(P, DC * PT))

    # layer-0 LN1 (gamma/beta folded into the weights) + fp8 split
    x0 = np.zeros((PT, D), np.float32)
    x0[:n] = _ln_np(base[:n])
    x0T = np.ascontiguousarray(
        x0.reshape(PT, DC, P).transpose(2, 1, 0).reshape(P, DC * PT))
    x80 = x0T.astype(E4)
    xr0 = (x0T - x80.astype(np.float32)).astype(E4)

    pidp = np.empty(PT, np.int64)
    pidp[:n] = pid
    pidp[n:] = -np.arange(1, PT - n + 1)

    runs = np.diff(np.concatenate(
        [[0], np.nonzero(np.diff(pidp))[0] + 1, [PT]]))
    if runs.max() > AW + 1:
        raise RuntimeError(
            f"patch run {runs.max()} exceeds attention window margin {AW + 1}")

    # per key tile j: WW-col query window [w0, w0+WW)
    m = np.zeros((NT, P, 2 * WW), np.float32)
    for j in range(NT):
        w0 = min(max(j * P - AW, 0), PT - WW)
        kk = pidp[j * P:(j + 1) * P]
        qq = pidp[w0:w0 + WW]
        blk = (kk[:, None] == qq[None, :]).astype(np.float32)
        m[j, :, 0:WW] = blk
        m[j, :, WW:2 * WW] = blk
    masks = np.ascontiguousarray(
        m.transpose(1, 0, 2).reshape(P, NT * 2 * WW)).astype(BF)
    return {"baseT": baseT, "x80": x80, "xr0": xr0, "masks": masks}


def _lay(w, nblk):
    """[nblk*128, C] -> [128, nblk*C] partition-major layout (no cast)."""
    C = w.shape[1]
    return np.ascontiguousarray(
        w.reshape(nblk, P, C).transpose(1, 0, 2).reshape(P, nblk * C))


def _fp8_pair(wlay):
    a = np.asarray(wlay * WS, np.float32)
    w8 = a.astype(E4)
    r8 = (a - w8.astype(np.float32)).astype(E4)
    return w8, r8


def kernel(**inputs):
    pid_all = np.asarray(inputs["patch_ids"])
    tokemb = np.asarray(inputs["tok_emb"], np.float32)

    ln0g = np.asarray(inputs["ln0_g"], np.float32)
    ln0b = np.asarray(inputs["ln0_b"], np.float32)
    shared = {}
    for l in range(L):
        g1 = np.asarray(inputs["ln1_g"][l], np.float32)
        n1 = np.asarray(inputs["ln1_b"][l], np.float32)
        g2 = np.asarray(inputs["ln2_g"][l], np.float32)
        n2 = np.asarray(inputs["ln2_b"][l], np.float32)
        Wq = np.asarray(inputs["Wq"][l], np.float32)
        Wk = np.asarray(inputs["Wk"][l], np.float32)
        Wv = np.asarray(inputs["Wv"][l], np.float32)
        Wo = np.asarray(inputs["Wo"][l], np.float32)
        W1 = np.asarray(inputs["W1"][l], np.float32)
        W2 = np.asarray(inputs["W2"][l], np.float32)

        bq_ = n1 @ Wq + np.asarray(inputs["bq"][l], np.float32)
        bv_eff = n1 @ Wv + np.asarray(inputs["bv"][l], np.float32)
        bo_ = bv_eff @ Wo + np.asarray(inputs["bo"][l], np.float32)
        b1_ = n2 @ W1 + np.asarray(inputs["b1"][l], np.float32)
        b2_ = np.asarray(inputs["b2"][l], np.float32)

        for nm, wmat, gg in (("wq", Wq, g1), ("wk", Wk, g1), ("wv", Wv, g1)):
            w8, r8 = _fp8_pair(_lay(gg[:, None] * wmat, DC))
            shared[f"{nm}8_{l}"] = w8
            shared[f"{nm}r_{l}"] = r8
        # W1 blocks ordered (fg, dc)
        w1lay = np.ascontiguousarray(
            (g2[:, None] * W1).reshape(DC, P, 4, 1024)
            .transpose(1, 2, 0, 3).reshape(P, DC * F))
        w8, r8 = _fp8_pair(w1lay)
        shared[f"w18_{l}"] = w8
        shared[f"w1r_{l}"] = r8
        shared[f"wo{l}"] = _lay(Wo, DC).astype(BF)
        # W2 blocks ordered (dg, fc): piece (dg,pc) = d-cols [dg*512,(dg+1)*512)
        w2lay = np.ascontiguousarray(
            W2.reshape(FC, P, 2, 512).transpose(1, 2, 0, 3).reshape(P, FC * D))
        w8, r8 = _fp8_pair(w2lay)
        shared[f"w28_{l}"] = w8
        shared[f"w2r_{l}"] = r8
        shared[f"bq{l}"] = np.ascontiguousarray(bq_)
        shared[f"bo{l}"] = np.ascontiguousarray(bo_)
        shared[f"b1{l}"] = np.ascontiguousarray(b1_)
        shared[f"b2{l}"] = np.ascontiguousarray(b2_)
    shared["b2r64"] = np.ascontiguousarray(
        (WS * np.stack([np.asarray(inputs["b2"][l], np.float32)
                        for l in range(L)]).reshape(1, L * D))).astype(BF)

    shards = []
    for b in range(B):
        pid = np.asarray(pid_all[b])
        bnd = np.nonzero(pid[1:] != pid[:-1])[0] + 1
        cand = bnd[(bnd >= S - PT) & (bnd <= PT)]
        if len(cand) == 0:
            raise RuntimeError("no patch boundary near S/2; cannot shard")
        s = int(cand[np.argmin(np.abs(cand - S // 2))])
        shards.append((b, 0, s))
        shards.append((b, s, S - s))

    in_maps = []
    for b, start, n in shards:
        m = dict(shared)
        m.update(_prep_core(inputs, tokemb, ln0g, ln0b, b, start, n))
        in_maps.append(m)

    nc = _get_nc()
    res = bass_utils.run_bass_kernel_spmd(nc, in_maps, core_ids=list(range(NCORES)))

    out = np.zeros((B, S, D), np.float32)
    for i, (b, start, n) in enumerate(shards):
        ht = res.results[i]["houtT"]
        hfull = ht.reshape(P, DC, PT).transpose(2, 1, 0).reshape(PT, D)
        out[b, start:start + n] = hfull[:n]
    return out


if __name__ == "__main__":
    _get_nc()
    print("built ok")


# revision 3
# speedup vs baseline: 1.0079x; 1.0079x over previous
"""BLT local encoder (2-layer transformer, patch-equality block-diagonal attention)
on 8 Trainium2 NeuronCores — v3.

Sharding: each of the 4 sequences is split at a patch boundary near S/2 into 2
independent shards -> 8 shards, one per core, no cross-core communication.

Structure:
- tok_emb gather on host (baseT = tok+pos+hash).
- LN gains folded into following weights on host; LN biases folded into
  projection biases; k-bias dropped (softmax-invariant); v-bias folded into
  the output-projection bias.  Kernel LN = (x-mu)*rstd only.
- K/Q/V/W1 projections run as error-compensated fp8 DoubleRow matmuls:
  weights W ~= (W8 + R8)/64 and activations x ~= x8 + r8, all fp8e4m3 at a
  shared scale, so the three products W8x8 + W8r8 + R8x8 accumulate in one
  PSUM group (the dropped R8r8 term is ~0.1%%*3.6%%).  DoubleRow contracts
  256 rows per matmul.  Precision matches bf16 (~2e-3 end to end).
- O-projection / W2 / attention-score path stay bf16.
- Attention: narrow windows — key tile j only sees queries within AW=32 of
  the tile (max patch run here is 11; host asserts run <= AW+1).  Softmax
  denominator merged into the ctx matmul via a ones-plane in Vext.
- QKVO loaded once per layer; W1/W2 streamed once per 384-token chunk.
- ctx overwrites QT per head (partition-disjoint); uT reuses KT's pool slot
  (KT is dead after the score matmuls).
"""

import numpy as np
import ml_dtypes

import concourse.bass as bass
import concourse.tile as tile
from concourse import bacc, bass_utils, mybir

F32 = mybir.dt.float32
F32R = mybir.dt.float32r
BF16 = mybir.dt.bfloat16
FP8 = mybir.dt.float8e4
AF = mybir.ActivationFunctionType
OP = mybir.AluOpType
DR = mybir.MatmulPerfMode.DoubleRow

B, S, D, H, F, L = 4, 2048, 1024, 16, 4096, 2
DH = D // H  # 64
DC = D // 128  # 8
FC = F // 128  # 32
EPS = 1e-5
SCALE = 1.0 / np.sqrt(DH)

P = 128
NT = 9           # token tiles per shard
PT = NT * P      # 1152
TC = 384         # token chunk
NCH = 3
NCORES = 8
BF = ml_dtypes.bfloat16
E4 = ml_dtypes.float8_e4m3
AW = 16          # attention window margin (max patch run must be <= AW+1)
WW = 128 + 2 * AW  # 192: per-key-tile query window width
WS = 64.0        # fp8 weight scale
IWS = 1.0 / WS


def _build():
    nc = bacc.Bacc("TRN2", target_bir_lowering=False, debug=False,
                   num_devices=NCORES)

    def din(name, shape, dt=F32):
        return nc.dram_tensor(name, shape, dt, kind="ExternalInput").ap()

    baseT = din("baseT", [P, DC * PT], F32R)   # LN0-applied residual (host)
    x80_d = din("x80", [P, DC * PT], FP8)      # layer-0 LN1 output (host)
    xr0_d = din("xr0", [P, DC * PT], FP8)      # its fp8 residual (host)
    masks_d = din("masks", [P, NT * 2 * WW], BF16)
    wq8, wqr, wk8, wkr, wv8, wvr = [], [], [], [], [], []
    w18, w1r, wo, w2 = [], [], [], []
    bq, bo, b1, b2 = [], [], [], []
    for l in range(L):
        wq8.append(din(f"wq8_{l}", [P, DC * D], FP8))
        wqr.append(din(f"wqr_{l}", [P, DC * D], FP8))
        wk8.append(din(f"wk8_{l}", [P, DC * D], FP8))
        wkr.append(din(f"wkr_{l}", [P, DC * D], FP8))
        wv8.append(din(f"wv8_{l}", [P, DC * D], FP8))
        wvr.append(din(f"wvr_{l}", [P, DC * D], FP8))
        w18.append(din(f"w18_{l}", [P, DC * F], FP8))
        w1r.append(din(f"w1r_{l}", [P, DC * F], FP8))
        wo.append(din(f"wo{l}", [P, DC * D], BF16))
        w2.append((din(f"w28_{l}", [P, FC * D], FP8),
                   din(f"w2r_{l}", [P, FC * D], FP8)))
        bq.append(din(f"bq{l}", [D]))
        bo.append(din(f"bo{l}", [D]))
        b1.append(din(f"b1{l}", [F]))
        b2.append(din(f"b2{l}", [D]))
    b2r64 = din("b2r64", [1, L * D], BF16)   # 64*b2 row-major per layer
    houtT = nc.dram_tensor("houtT", [P, DC * PT], F32R, kind="ExternalOutput").ap()

    with tile.TileContext(nc) as tc:
        with (
            nc.allow_low_precision(reason="fp8/bf16 compensated path within budget"),
            tc.tile_pool(name="pers", bufs=1) as pers,
            tc.tile_pool(name="x8p", bufs=1) as x8p,
            tc.tile_pool(name="kup", bufs=1) as kup,
            tc.tile_pool(name="qtp", bufs=1) as qtp,
            tc.tile_pool(name="vxp", bufs=1) as vxp,
            tc.tile_pool(name="estp", bufs=2) as estp,
            tc.tile_pool(name="wp8", bufs=6) as wp8,
            tc.tile_pool(name="wpb", bufs=3) as wpb,
            tc.tile_pool(name="lnp", bufs=4) as lnp,
            tc.tile_pool(name="rmp", bufs=2) as rmp,
            tc.tile_pool(name="small", bufs=2) as small,
            tc.tile_pool(name="nrmp", bufs=2) as nrmp,
            tc.tile_pool(name="pp", bufs=8, space="PSUM") as pp,
        ):
            # Tiles allocated up front; the DMAs that fill cpack / masks /
            # hT are emitted late (inside layer 0) so the DMA queue serves
            # layer-0's critical path (x80/xr0/wk) first.
            hT = pers.tile([P, DC * PT], F32R, tag="hT")
            hT3 = hT.rearrange("p (c t) -> p c t", c=DC)
            cpack = pers.tile([P, 132], F32, tag="cpack")
            nc.vector.memset(cpack[:, 0:1], 1.0)
            nc.vector.memset(cpack[0:1, 1:2], EPS)
            bcol = [cpack[:, 18 + 56 * l:18 + 56 * l + 56] for l in range(L)]
            eps_t = cpack[0:1, 1:2]
            ones_col = pers.tile([P, 1], F32R, tag="ones_col")
            nc.vector.tensor_copy(ones_col, cpack[:, 0:1])
            ones_bf = pers.tile([P, 1], BF16, tag="ones_bf")
            nc.vector.tensor_copy(ones_bf, cpack[:, 0:1])
            mk = pers.tile([P, NT * 2 * WW], BF16, tag="mk")
            ones_row = pers.tile([1, TC], BF16, tag="ones_row")
            nc.vector.memset(ones_row, 1.0)
            b2row = pers.tile([1, L * D], BF16, tag="b2row")

            def emit_late_dmas():
                nc.sync.dma_start(out=b2row, in_=b2r64)
                for l in range(L):
                    b0 = 18 + 56 * l
                    for i, v in enumerate((bq[l], bo[l], b2[l])):
                        nc.sync.dma_start(
                            out=cpack[:, b0 + 8 * i:b0 + 8 * i + 8],
                            in_=v.rearrange("(c p) -> p c", p=P))
                    nc.sync.dma_start(out=cpack[:, b0 + 24:b0 + 56],
                                      in_=b1[l].rearrange("(c p) -> p c", p=P))
                nc.sync.dma_start(out=mk, in_=masks_d)
                baseT3 = baseT.rearrange("p (c t) -> p c t", c=DC)
                for ci in range(NCH):
                    t0 = ci * TC
                    nc.sync.dma_start(out=hT3[:, :, t0:t0 + TC],
                                      in_=baseT3[:, :, t0:t0 + TC])

            def ln_chunk(ci, out_pair=None, gb=None):
                """LayerNorm over features (partitions) for token chunk ci.
                gb=(g,b): write (x-mu)*rstd*g+b back into hT (float32).
                out_pair=(x8,xr): write fp8 value + fp8 residual."""
                t0 = ci * TC
                ps1 = pp.tile([1, TC], F32, tag="mm", name=f"lns1_{ci}")
                ps2 = pp.tile([1, TC], F32, tag="mm", name=f"lns2_{ci}")
                for dc in range(DC):
                    hsl = hT[:, dc * PT + t0:dc * PT + t0 + TC]
                    nc.tensor.matmul(ps1, lhsT=ones_col, rhs=hsl,
                                     start=(dc == 0), stop=(dc == DC - 1))
                    sq = lnp.tile([P, TC], BF16, tag="lnsq", name=f"lnsq{dc}")
                    nc.scalar.square(sq, hsl)
                    nc.tensor.matmul(ps2, lhsT=ones_bf, rhs=sq,
                                     start=(dc == 0), stop=(dc == DC - 1))
                mean = small.tile([1, TC], F32, tag="smf", name="mean")
                var = small.tile([1, TC], F32, tag="smf", name="var")
                rstd = small.tile([1, TC], BF16, tag="smb", name="rstd")
                mr = small.tile([1, TC], BF16, tag="smb", name="mr")
                nc.vector.tensor_scalar_mul(mean, ps1, 1.0 / D)
                nc.vector.tensor_mul(var, mean, mean)
                nc.vector.scalar_tensor_tensor(var, ps2, 1.0 / D, var,
                                               op0=OP.mult, op1=OP.subtract)
                nc.scalar.activation(var, var, AF.Sqrt, bias=eps_t)
                nc.vector.reciprocal(rstd, var)
                nc.vector.tensor_mul(mr, mean, rstd)
                RM = rmp.tile([P, 2 * TC], BF16, tag="lnRM")
                nc.gpsimd.partition_broadcast(RM[:, 0:TC], rstd)
                nc.gpsimd.partition_broadcast(RM[:, TC:2 * TC], mr)
                for dc in range(DC):
                    hsl = hT[:, dc * PT + t0:dc * PT + t0 + TC]
                    d1 = lnp.tile([P, TC], F32, tag="lnt", name=f"lnd{dc}")
                    nc.vector.tensor_mul(d1, hsl, RM[:, 0:TC])
                    if gb is not None:
                        d2 = lnp.tile([P, TC], F32, tag="lnt", name=f"lnd2_{dc}")
                        nc.vector.tensor_sub(d2, d1, RM[:, TC:2 * TC])
                        gcol, bc = gb
                        nc.vector.tensor_scalar(
                            hsl, d2, gcol[:, dc:dc + 1], bc[:, dc:dc + 1],
                            op0=OP.mult, op1=OP.add)
                    else:
                        t = lnp.tile([P, TC], F32, tag="lnt", name=f"lnx{dc}")
                        nc.vector.tensor_sub(t, d1, RM[:, TC:2 * TC])
                        x8sl = out_pair[0][:, dc * PT + t0:dc * PT + t0 + TC]
                        r8sl = out_pair[1][:, dc * PT + t0:dc * PT + t0 + TC]
                        nc.scalar.activation(x8sl, t, AF.Copy)
                        nc.vector.tensor_sub(r8sl, t, x8sl)



            def dr_accum(ps, w8v, wrv, xa3, xr3a, c0, cw, t0, tw):
                """12 DoubleRow MMs: psum += (W8+R8)^T(x8+r8) minus R8r8,
                contraction over all 8 dc blocks (4 pairs x 3 terms)."""
                n = 0
                for dcp in range(4):
                    hf, lc = dcp // 2, (2 * dcp) % 4
                    w8sl = w8v[hf][:, lc:lc + 2, c0:c0 + cw]
                    wrsl = wrv[hf][:, lc:lc + 2, c0:c0 + cw]
                    x8sl = xa3[:, 2 * dcp:2 * dcp + 2, t0:t0 + tw]
                    xrsl = xr3a[:, 2 * dcp:2 * dcp + 2, t0:t0 + tw]
                    for lh, rh in ((w8sl, x8sl), (w8sl, xrsl), (wrsl, x8sl)):
                        nc.tensor.matmul(ps, lhsT=lh, rhs=rh,
                                         start=(n == 0), stop=(n == 11),
                                         perf_mode=DR)
                        n += 1

            def dr_accum_tok(ps, w8v, wrv, xa3, xr3a, c0, cw, tt):
                """Token-major variant: lhsT = activation pairs."""
                n = 0
                for dcp in range(4):
                    hf, lc = dcp // 2, (2 * dcp) % 4
                    x8sl = xa3[:, 2 * dcp:2 * dcp + 2, tt * 128:tt * 128 + 128]
                    xrsl = xr3a[:, 2 * dcp:2 * dcp + 2, tt * 128:tt * 128 + 128]
                    w8sl = w8v[hf][:, lc:lc + 2, c0:c0 + cw]
                    wrsl = wrv[hf][:, lc:lc + 2, c0:c0 + cw]
                    for lh, rh in ((x8sl, w8sl), (xrsl, w8sl), (x8sl, wrsl)):
                        nc.tensor.matmul(ps, lhsT=lh, rhs=rh,
                                         start=(n == 0), stop=(n == 11),
                                         perf_mode=DR)
                        n += 1

            def load_w8(dram8, dramr, tag, l):
                """Load an fp8 [D,D]-class matrix + residual as 2 half tiles
                each, returning ([P,4,1024] views x 2 halves) for both."""
                v8, vr = [], []
                for hf in range(2):
                    t8 = wp8.tile([P, 4 * D], FP8, tag="w8", name=f"{tag}8_{l}_{hf}")
                    nc.sync.dma_start(out=t8, in_=dram8[:, hf * 4 * D:(hf + 1) * 4 * D])
                    v8.append(t8.rearrange("p (c d) -> p c d", c=4))
                    tr = wp8.tile([P, 4 * D], FP8, tag="w8", name=f"{tag}r_{l}_{hf}")
                    nc.sync.dma_start(out=tr, in_=dramr[:, hf * 4 * D:(hf + 1) * 4 * D])
                    vr.append(tr.rearrange("p (c d) -> p c d", c=4))
                return v8, vr

            # ---------- layers ----------
            for l in range(L):
                # ---- LN1 -> x8 + residual (fp8, full shard) ----
                # layer 0's LN1 is input-only: host ships it pre-computed
                x8 = x8p.tile([P, DC * PT], FP8, tag="x8", name=f"x8_{l}")
                xr = x8p.tile([P, DC * PT], FP8, tag="xr", name=f"xr_{l}")
                if l == 0:
                    x8c = x8.rearrange("p (c t) -> p c t", c=DC)
                    xrc = xr.rearrange("p (c t) -> p c t", c=DC)
                    x80c = x80_d.rearrange("p (c t) -> p c t", c=DC)
                    xr0c = xr0_d.rearrange("p (c t) -> p c t", c=DC)
                    for ci in range(NCH):
                        t0 = ci * TC
                        nc.sync.dma_start(out=x8c[:, :, t0:t0 + TC],
                                          in_=x80c[:, :, t0:t0 + TC])
                        nc.sync.dma_start(out=xrc[:, :, t0:t0 + TC],
                                          in_=xr0c[:, :, t0:t0 + TC])
                else:
                    for ci in range(NCH):
                        ln_chunk(ci, out_pair=(x8, xr))
                x83 = x8.rearrange("p (c t) -> p c t", c=DC)
                xr3 = xr.rearrange("p (c t) -> p c t", c=DC)

                # ---- K (feature-major KT) and V (token-major Vext) ----
                KT = kup.tile([P, DC * PT], BF16, tag="ku", name=f"KT{l}")
                Vx = vxp.tile([P, 65, NT * H], BF16, tag="vx", name=f"Vx{l}")
                nc.vector.memset(Vx[:, 64:65, :], 1.0)

                wkb8, wkbr = load_w8(wk8[l], wkr[l], "wk", l)
                for ci in range(NCH):
                    t0 = ci * TC
                    for og in range(2):
                        pss = [pp.tile([P, TC], F32, tag="mm", name=f"psk{i}")
                               for i in range(4)]
                        for o4 in range(4):
                            oc = og * 4 + o4
                            dr_accum(pss[o4], wkb8, wkbr, x83, xr3,
                                     oc * 128, 128, t0, TC)
                        for o4 in range(4):
                            oc = og * 4 + o4
                            nc.scalar.activation(
                                KT[:, oc * PT + t0:oc * PT + t0 + TC], pss[o4],
                                AF.Copy, scale=IWS)

                wvb8, wvbr = load_w8(wv8[l], wvr[l], "wv", l)
                for tt in range(NT):
                    for nh in range(2):
                        psv = pp.tile([P, 512], F32, tag="mm", name=f"psv{tt}_{nh}")
                        dr_accum_tok(psv, wvb8, wvbr, x83, xr3,
                                     nh * 512, 512, tt)
                        nc.scalar.activation(
                            Vx[:, 0:64, tt * H + nh * 8:tt * H + nh * 8 + 8],
                            psv.rearrange("p (h x) -> p x h", h=8),
                            AF.Copy, scale=IWS)

                # ---- Q (feature-major QT, bias bq') ----
                QT = qtp.tile([P, DC * PT], BF16, tag="qt", name=f"QT{l}")
                wqb8, wqbr = load_w8(wq8[l], wqr[l], "wq", l)
                if l == 0:
                    emit_late_dmas()
                for ci in range(NCH):
                    t0 = ci * TC
                    for og in range(2):
                        psq = [pp.tile([P, TC], F32, tag="mm", name=f"psq{i}")
                               for i in range(4)]
                        for o4 in range(4):
                            oc = og * 4 + o4
                            dr_accum(psq[o4], wqb8, wqbr, x83, xr3,
                                     oc * 128, 128, t0, TC)
                        for o4 in range(4):
                            oc = og * 4 + o4
                            nc.scalar.activation(
                                QT[:, oc * PT + t0:oc * PT + t0 + TC], psq[o4],
                                AF.Identity, bias=bcol[l][:, oc:oc + 1], scale=IWS)

                # ---- attention; ctx overwrites QT regions per head ----
                ests = {}

                def w0_of(j):
                    return min(max(j * 128 - AW, 0), PT - WW)

                def attn_scores(i):
                    est = estp.tile([P, NT * 384], BF16, tag="est", name=f"est{i}")
                    ests[i] = est
                    for j in range(NT):
                        w0 = w0_of(j)
                        for ho in range(2):
                            po = ho * 64
                            pst = pp.tile([P, 384], F32, tag="mm", name=f"pst{j}_{ho}")
                            nc.tensor.matmul(
                                pst[:, 0:WW],
                                lhsT=KT[po:po + 64, i * PT + j * 128:i * PT + j * 128 + 128],
                                rhs=QT[po:po + 64, i * PT + w0:i * PT + w0 + WW],
                                start=True, stop=True)
                            esl = est[:, j * 384 + ho * WW:j * 384 + (ho + 1) * WW]
                            nc.scalar.activation(esl, pst[:, 0:WW], AF.Exp,
                                                 scale=float(SCALE))
                        ej = est[:, j * 384:j * 384 + 2 * WW]
                        nc.vector.tensor_mul(ej, ej, mk[:, j * 2 * WW:(j + 1) * 2 * WW])

                def attn_ctx(i):
                    est = ests.pop(i)
                    for ho in range(2):
                        h = 2 * i + ho
                        po = ho * 64
                        for c in range(NCH):
                            psc = pp.tile([P, TC], F32, tag="mm", name=f"psc{h}_{c}")
                            mms = []
                            for qi in range(3):
                                qt = 3 * c + qi
                                w0c = w0_of(qt)
                                mms.append((qi * 128, 128, qt, qt * 128 - w0c, True))
                                if qt - 1 >= 0:
                                    w0e = w0_of(qt - 1)
                                    mms.append((qi * 128, AW, qt - 1,
                                                qt * 128 - w0e, False))
                                if qt + 1 < NT:
                                    w0e = w0_of(qt + 1)
                                    mms.append((qi * 128 + 128 - AW, AW, qt + 1,
                                                (qt + 1) * 128 - AW - w0e, False))
                            for mi, (pcol, wdt, j, ecol, st) in enumerate(mms):
                                nc.tensor.matmul(
                                    psc[0:65, pcol:pcol + wdt],
                                    lhsT=Vx[:, 0:65, j * H + h:j * H + h + 1],
                                    rhs=est[:, j * 384 + ho * WW + ecol:
                                            j * 384 + ho * WW + ecol + wdt],
                                    start=st, stop=(mi == len(mms) - 1),
                                    skip_group_check=True)
                            den = nrmp.tile([1, TC], BF16, tag="den", name=f"den{h}_{c}")
                            rnm = nrmp.tile([64, TC], BF16, tag="rnm", name=f"rnm{h}_{c}")
                            nc.vector.reciprocal(den, psc[64:65, :])
                            nc.gpsimd.partition_broadcast(rnm, den)
                            nc.vector.tensor_mul(
                                QT[po:po + 64, i * PT + c * TC:i * PT + c * TC + TC],
                                psc[0:64, :], rnm)

                attn_scores(0)
                for i in range(1, H // 2):
                    attn_scores(i)
                    attn_ctx(i - 1)
                attn_ctx(H // 2 - 1)

                # ---- O-projection + residual (ctx lives in QT; bf16) ----
                wob = []
                for hf in range(2):
                    t = wpb.tile([P, 4 * D], BF16, tag="w", name=f"wob{l}_{hf}")
                    nc.sync.dma_start(out=t, in_=wo[l][:, hf * 4 * D:(hf + 1) * 4 * D])
                    wob.append(t)
                for ci in range(NCH):
                    t0 = ci * TC
                    for og in range(2):
                        pso = [pp.tile([P, TC], F32, tag="mm", name=f"pso{i}")
                               for i in range(4)]
                        for di in range(DC):
                            wb = wob[di // 4]
                            for o4 in range(4):
                                do_ = og * 4 + o4
                                nc.tensor.matmul(
                                    pso[o4],
                                    lhsT=wb[:, (di % 4) * D + do_ * 128:(di % 4) * D + do_ * 128 + 128],
                                    rhs=QT[:, di * PT + t0:di * PT + t0 + TC],
                                    start=(di == 0), stop=(di == DC - 1))
                        for o4 in range(4):
                            do_ = og * 4 + o4
                            hsl = hT[:, do_ * PT + t0:do_ * PT + t0 + TC]
                            nc.vector.scalar_tensor_tensor(
                                hsl, pso[o4], bcol[l][:, 8 + do_:8 + do_ + 1], hsl,
                                op0=OP.add, op1=OP.add)

                # ---- LN2 -> x8/xr (reuses the x8p slots) ----
                x28 = x8p.tile([P, DC * PT], FP8, tag="x8", name=f"x28_{l}")
                x2r = x8p.tile([P, DC * PT], FP8, tag="xr", name=f"x2r_{l}")
                for ci in range(NCH):
                    ln_chunk(ci, out_pair=(x28, x2r))
                x283 = x28.rearrange("p (c t) -> p c t", c=DC)
                x2r3 = x2r.rearrange("p (c t) -> p c t", c=DC)

                # ---- FFN: W1 and W2 both fp8-DR, compensated uT ----
                for ci in range(NCH):
                    t0 = ci * TC
                    # u8 in first half, its fp8 residual ur in second half
                    uT = kup.tile([P, 2 * FC * TC], FP8, tag="ku", name=f"uT{l}_{ci}")
                    UR = FC * TC
                    for fg in range(4):
                        p8, pr = [], []
                        for hf in range(2):
                            t8 = wp8.tile([P, 4096], FP8, tag="w8",
                                          name=f"w18p{l}_{ci}_{fg}_{hf}")
                            nc.sync.dma_start(
                                out=t8,
                                in_=w18[l][:, fg * 8192 + hf * 4096:fg * 8192 + (hf + 1) * 4096])
                            p8.append(t8.rearrange("p (c d) -> p c d", c=4))
                            tr = wp8.tile([P, 4096], FP8, tag="w8",
                                          name=f"w1rp{l}_{ci}_{fg}_{hf}")
                            nc.sync.dma_start(
                                out=tr,
                                in_=w1r[l][:, fg * 8192 + hf * 4096:fg * 8192 + (hf + 1) * 4096])
                            pr.append(tr.rearrange("p (c d) -> p c d", c=4))
                        for fgi in range(2):
                            psf = [pp.tile([P, TC], F32, tag="mm", name=f"psf{i}")
                                   for i in range(4)]
                            for f4 in range(4):
                                fcl = fgi * 4 + f4
                                dr_accum(psf[f4], p8, pr, x283, x2r3,
                                         fcl * 128, 128, t0, TC)
                            for f4 in range(4):
                                fc = fg * 8 + fgi * 4 + f4
                                gt = lnp.tile([P, TC], F32, tag="lnt",
                                              name=f"gt{fc}")
                                nc.scalar.activation(
                                    gt, psf[f4], AF.Gelu,
                                    bias=bcol[l][:, 24 + fc:24 + fc + 1], scale=IWS)
                                u8sl = uT[:, fc * TC:(fc + 1) * TC]
                                ursl = uT[:, UR + fc * TC:UR + (fc + 1) * TC]
                                if fc % 2 == 0:
                                    nc.scalar.activation(u8sl, gt, AF.Copy)
                                else:
                                    nc.vector.tensor_copy(u8sl, gt)
                                nc.vector.tensor_sub(ursl, gt, u8sl)
                    def w2_piece(psh, nmm, dg, pc):
                        wb8 = wp8.tile([P, 4096], FP8, tag="w8",
                                       name=f"w28p{l}_{ci}_{dg}_{pc}")
                        nc.sync.dma_start(
                            out=wb8,
                            in_=w2[l][0][:, dg * 16384 + pc * 4096:dg * 16384 + (pc + 1) * 4096])
                        wbr = wp8.tile([P, 4096], FP8, tag="w8",
                                       name=f"w2rp{l}_{ci}_{dg}_{pc}")
                        nc.sync.dma_start(
                            out=wbr,
                            in_=w2[l][1][:, dg * 16384 + pc * 4096:dg * 16384 + (pc + 1) * 4096])
                        w83 = wb8.rearrange("p (c d) -> p c d", c=8)
                        wr3 = wbr.rearrange("p (c d) -> p c d", c=8)
                        u83 = uT.rearrange("p (c t) -> p c t", c=2 * FC)
                        for fp_ in range(4):
                            fc = pc * 8 + 2 * fp_
                            u8sl = u83[:, fc:fc + 2, :]
                            ursl = u83[:, FC + fc:FC + fc + 2, :]
                            for o4 in range(4):
                                w8sl = w83[:, 2 * fp_:2 * fp_ + 2,
                                           o4 * 128:o4 * 128 + 128]
                                wrsl = wr3[:, 2 * fp_:2 * fp_ + 2,
                                           o4 * 128:o4 * 128 + 128]
                                for lh, rh in ((w8sl, u8sl), (w8sl, ursl),
                                               (wrsl, u8sl)):
                                    nc.tensor.matmul(
                                        psh[o4], lhsT=lh, rhs=rh,
                                        start=(nmm[o4] == 0), stop=False,
                                        perf_mode=DR)
                                    nmm[o4] += 1

                    for dg in range(2):
                        psh = [pp.tile([P, TC], F32, tag="mm", name=f"psh{i}")
                               for i in range(4)]
                        nmm = [0] * 4
                        for pc in range(4):
                            w2_piece(psh, nmm, dg, pc)
                        for o4 in range(4):
                            do_ = dg * 4 + o4
                            # + 64*b2 via a rank-1 matmul, closing the group
                            nc.tensor.matmul(
                                psh[o4],
                                lhsT=b2row[:, l * D + do_ * 128:l * D + do_ * 128 + 128],
                                rhs=ones_row,
                                start=False, stop=True)
                        for o4 in range(4):
                            do_ = dg * 4 + o4
                            hsl = hT[:, do_ * PT + t0:do_ * PT + t0 + TC]
                            nc.vector.scalar_tensor_tensor(
                                hsl, psh[o4], IWS, hsl,
                                op0=OP.mult, op1=OP.add)

            # chunked output DMA: chunk c leaves as soon as its last residual
            # add lands
            houtT3 = houtT.rearrange("p (c t) -> p c t", c=DC)
            for ci in range(NCH):
                t0 = ci * TC
                nc.sync.dma_start(out=houtT3[:, :, t0:t0 + TC],
                                  in_=hT3[:, :, t0:t0 + TC])

    nc.compile()
    return nc


_NC_CACHE = {}


def _get_nc():
    if "nc" not in _NC_CACHE:
        _NC_CACHE["nc"] = _build()
    return _NC_CACHE["nc"]


def _ln_np(x, g=None, bta=None):
    mu = x.mean(-1, keepdims=True)
    var = ((x - mu) ** 2).mean(-1, keepdims=True)
    y = (x - mu) / np.sqrt(var + EPS)
    if g is not None:
        y = y * g + bta
    return y.astype(np.float32)


def _prep_core(inputs, tokemb_f32, ln0g, ln0b, b, start, n):
    """Per-core in_map entries that depend on the shard."""
    ids = np.asarray(inputs["input_ids"][b, start:start + n]).astype(np.int64)
    pid = np.asarray(inputs["patch_ids"][b, start:start + n]).astype(np.int64)
    pos_emb = np.asarray(inputs["pos_emb"], np.float32)
    hashes = np.asarray(inputs["hash_embeddings"], np.float32)

    base = np.zeros((PT, D), np.float32)
    emb = tokemb_f32[ids] + pos_emb[start:start + n] + hashes[b, start:start + n]
    base[:n] = _ln_np(emb, ln0g, ln0b)
    baseT = np.ascontiguousarray(
        base.reshape(PT, DC, P).transpose(2, 1, 0).reshape(P, DC * PT))

    # layer-0 LN1 (gamma/beta folded into the weights) + fp8 split
    x0 = np.zeros((PT, D), np.float32)
    x0[:n] = _ln_np(base[:n])
    x0T = np.ascontiguousarray(
        x0.reshape(PT, DC, P).transpose(2, 1, 0).reshape(P, DC * PT))
    x80 = x0T.astype(E4)
    xr0 = (x0T - x80.astype(np.float32)).astype(E4)

    pidp = np.empty(PT, np.int64)
    pidp[:n] = pid
    pidp[n:] = -np.arange(1, PT - n + 1)

    runs = np.diff(np.concatenate(
        [[0], np.nonzero(np.diff(pidp))[0] + 1, [PT]]))
    if runs.max() > AW + 1:
        raise RuntimeError(
            f"patch run {runs.max()} exceeds attention window margin {AW + 1}")

    # per key tile j: WW-col query window [w0, w0+WW)
    m = np.zeros((NT, P, 2 * WW), np.float32)
    for j in range(NT):
        w0 = min(max(j * P - AW, 0), PT - WW)
        kk = pidp[j * P:(j + 1) * P]
        qq = pidp[w0:w0 + WW]
        blk = (kk[:, None] == qq[None, :]).astype(np.float32)
        m[j, :, 0:WW] = blk
        m[j, :, WW:2 * WW] = blk
    masks = np.ascontiguousarray(
        m.transpose(1, 0, 2).reshape(P, NT * 2 * WW)).astype(BF)
    return {"baseT": baseT, "x80": x80, "xr0": xr0, "masks": masks}


def _lay(w, nblk):
    """[nblk*128, C] -> [128, nblk*C] partition-major layout (no cast)."""
    C = w.shape[1]
    return np.ascontiguousarray(
        w.reshape(nblk, P, C).transpose(1, 0, 2).reshape(P, nblk * C))


def _fp8_pair(wlay):
    a = np.asarray(wlay * WS, np.float32)
    w8 = a.astype(E4)
    r8 = (a - w8.astype(np.float32)).astype(E4)
    return w8, r8


def kernel(**inputs):
    pid_all = np.asarray(inputs["patch_ids"])
    tokemb = np.asarray(inputs["tok_emb"], np.float32)

    ln0g = np.asarray(inputs["ln0_g"], np.float32)
    ln0b = np.asarray(inputs["ln0_b"], np.float32)
    shared = {}
    for l in range(L):
        g1 = np.asarray(inputs["ln1_g"][l], np.float32)
        n1 = np.asarray(inputs["ln1_b"][l], np.float32)
        g2 = np.asarray(inputs["ln2_g"][l], np.float32)
        n2 = np.asarray(inputs["ln2_b"][l], np.float32)
        Wq = np.asarray(inputs["Wq"][l], np.float32)
        Wk = np.asarray(inputs["Wk"][l], np.float32)
        Wv = np.asarray(inputs["Wv"][l], np.float32)
        Wo = np.asarray(inputs["Wo"][l], np.float32)
        W1 = np.asarray(inputs["W1"][l], np.float32)
        W2 = np.asarray(inputs["W2"][l], np.float32)

        bq_ = n1 @ Wq + np.asarray(inputs["bq"][l], np.float32)
        bv_eff = n1 @ Wv + np.asarray(inputs["bv"][l], np.float32)
        bo_ = bv_eff @ Wo + np.asarray(inputs["bo"][l], np.float32)
        b1_ = n2 @ W1 + np.asarray(inputs["b1"][l], np.float32)
        b2_ = np.asarray(inputs["b2"][l], np.float32)

        for nm, wmat, gg in (("wq", Wq, g1), ("wk", Wk, g1), ("wv", Wv, g1)):
            w8, r8 = _fp8_pair(_lay(gg[:, None] * wmat, DC))
            shared[f"{nm}8_{l}"] = w8
            shared[f"{nm}r_{l}"] = r8
        # W1 blocks ordered (fg, dc)
        w1lay = np.ascontiguousarray(
            (g2[:, None] * W1).reshape(DC, P, 4, 1024)
            .transpose(1, 2, 0, 3).reshape(P, DC * F))
        w8, r8 = _fp8_pair(w1lay)
        shared[f"w18_{l}"] = w8
        shared[f"w1r_{l}"] = r8
        shared[f"wo{l}"] = _lay(Wo, DC).astype(BF)
        # W2 blocks ordered (dg, fc): piece (dg,pc) = d-cols [dg*512,(dg+1)*512)
        w2lay = np.ascontiguousarray(
            W2.reshape(FC, P, 2, 512).transpose(1, 2, 0, 3).reshape(P, FC * D))
        w8, r8 = _fp8_pair(w2lay)
        shared[f"w28_{l}"] = w8
        shared[f"w2r_{l}"] = r8
        shared[f"bq{l}"] = np.ascontiguousarray(bq_)
        shared[f"bo{l}"] = np.ascontiguousarray(bo_)
        shared[f"b1{l}"] = np.ascontiguousarray(b1_)
        shared[f"b2{l}"] = np.ascontiguousarray(b2_)
    shared["b2r64"] = np.ascontiguousarray(
        (WS * np.stack([np.asarray(inputs["b2"][l], np.float32)
                        for l in range(L)]).reshape(1, L * D))).astype(BF)

    shards = []
    for b in range(B):
        pid = np.asarray(pid_all[b])
        bnd = np.nonzero(pid[1:] != pid[:-1])[0] + 1
        cand = bnd[(bnd >= S - PT) & (bnd <= PT)]
        if len(cand) == 0:
            raise RuntimeError("no patch boundary near S/2; cannot shard")
        s = int(cand[np.argmin(np.abs(cand - S // 2))])
        shards.append((b, 0, s))
        shards.append((b, s, S - s))

    in_maps = []
    for b, start, n in shards:
        m = dict(shared)
        m.update(_prep_core(inputs, tokemb, ln0g, ln0b, b, start, n))
        in_maps.append(m)

    nc = _get_nc()
    res = bass_utils.run_bass_kernel_spmd(nc, in_maps, core_ids=list(range(NCORES)))

    out = np.zeros((B, S, D), np.float32)
    for i, (b, start, n) in enumerate(shards):
        ht = res.results[i]["houtT"]
        hfull = ht.reshape(P, DC, PT).transpose(2, 1, 0).reshape(PT, D)
        out[b, start:start + n] = hfull[:n]
    return out


if __name__ == "__main__":
    _get_nc()
    print("built ok")


# revision 4
# speedup vs baseline: 1.0081x; 1.0001x over previous
"""BLT local encoder (2-layer transformer, patch-equality block-diagonal attention)
on 8 Trainium2 NeuronCores — v3.

Sharding: each of the 4 sequences is split at a patch boundary near S/2 into 2
independent shards -> 8 shards, one per core, no cross-core communication.

Structure:
- tok_emb gather on host (baseT = tok+pos+hash).
- LN gains folded into following weights on host; LN biases folded into
  projection biases; k-bias dropped (softmax-invariant); v-bias folded into
  the output-projection bias.  Kernel LN = (x-mu)*rstd only.
- K/Q/V/W1 projections run as error-compensated fp8 DoubleRow matmuls:
  weights W ~= (W8 + R8)/64 and activations x ~= x8 + r8, all fp8e4m3 at a
  shared scale, so the three products W8x8 + W8r8 + R8x8 accumulate in one
  PSUM group (the dropped R8r8 term is ~0.1%%*3.6%%).  DoubleRow contracts
  256 rows per matmul.  Precision matches bf16 (~2e-3 end to end).
- O-projection / W2 / attention-score path stay bf16.
- Attention: narrow windows — key tile j only sees queries within AW=32 of
  the tile (max patch run here is 11; host asserts run <= AW+1).  Softmax
  denominator merged into the ctx matmul via a ones-plane in Vext.
- QKVO loaded once per layer; W1/W2 streamed once per 384-token chunk.
- ctx overwrites QT per head (partition-disjoint); uT reuses KT's pool slot
  (KT is dead after the score matmuls).
"""

import numpy as np
import ml_dtypes

import concourse.bass as bass
import concourse.tile as tile
from concourse import bacc, bass_utils, mybir

F32 = mybir.dt.float32
F32R = mybir.dt.float32r
BF16 = mybir.dt.bfloat16
FP8 = mybir.dt.float8e4
AF = mybir.ActivationFunctionType
OP = mybir.AluOpType
DR = mybir.MatmulPerfMode.DoubleRow

B, S, D, H, F, L = 4, 2048, 1024, 16, 4096, 2
DH = D // H  # 64
DC = D // 128  # 8
FC = F // 128  # 32
EPS = 1e-5
SCALE = 1.0 / np.sqrt(DH)

P = 128
NT = 9           # token tiles per shard
PT = NT * P      # 1152
TC = 384         # token chunk
NCH = 3
NCORES = 8
BF = ml_dtypes.bfloat16
E4 = ml_dtypes.float8_e4m3
AW = 16          # attention window margin (max patch run must be <= AW+1)
WW = 128 + 2 * AW  # 192: per-key-tile query window width
WS = 64.0        # fp8 weight scale
IWS = 1.0 / WS


def _build():
    nc = bacc.Bacc("TRN2", target_bir_lowering=False, debug=False,
                   num_devices=NCORES)

    def din(name, shape, dt=F32):
        return nc.dram_tensor(name, shape, dt, kind="ExternalInput").ap()

    baseT = din("baseT", [P, DC * PT], F32R)   # LN0-applied residual (host)
    x80_d = din("x80", [P, DC * PT], FP8)      # layer-0 LN1 output (host)
    xr0_d = din("xr0", [P, DC * PT], FP8)      # its fp8 residual (host)
    masks_d = din("masks", [P, NT * 2 * WW], BF16)
    wq8, wqr, wk8, wkr, wv8, wvr = [], [], [], [], [], []
    w18, w1r, wo, w2 = [], [], [], []
    bq, bo, b1, b2 = [], [], [], []
    for l in range(L):
        wq8.append(din(f"wq8_{l}", [P, DC * D], FP8))
        wqr.append(din(f"wqr_{l}", [P, DC * D], FP8))
        wk8.append(din(f"wk8_{l}", [P, DC * D], FP8))
        wkr.append(din(f"wkr_{l}", [P, DC * D], FP8))
        wv8.append(din(f"wv8_{l}", [P, DC * D], FP8))
        wvr.append(din(f"wvr_{l}", [P, DC * D], FP8))
        w18.append(din(f"w18_{l}", [P, DC * F], FP8))
        w1r.append(din(f"w1r_{l}", [P, DC * F], FP8))
        wo.append(din(f"wo{l}", [P, DC * D], BF16))
        w2.append((din(f"w28_{l}", [P, FC * D], FP8),
                   din(f"w2r_{l}", [P, FC * D], FP8)))
        bq.append(din(f"bq{l}", [D]))
        bo.append(din(f"bo{l}", [D]))
        b1.append(din(f"b1{l}", [F]))
        b2.append(din(f"b2{l}", [D]))
    b2r64 = din("b2r64", [1, L * D], BF16)   # 64*b2 row-major per layer
    houtT = nc.dram_tensor("houtT", [P, DC * PT], F32R, kind="ExternalOutput").ap()

    with tile.TileContext(nc) as tc:
        with (
            nc.allow_low_precision(reason="fp8/bf16 compensated path within budget"),
            tc.tile_pool(name="pers", bufs=1) as pers,
            tc.tile_pool(name="x8p", bufs=1) as x8p,
            tc.tile_pool(name="kup", bufs=1) as kup,
            tc.tile_pool(name="qtp", bufs=1) as qtp,
            tc.tile_pool(name="vxp", bufs=1) as vxp,
            tc.tile_pool(name="estp", bufs=2) as estp,
            tc.tile_pool(name="wp8", bufs=6) as wp8,
            tc.tile_pool(name="wpb", bufs=3) as wpb,
            tc.tile_pool(name="lnp", bufs=4) as lnp,
            tc.tile_pool(name="rmp", bufs=2) as rmp,
            tc.tile_pool(name="small", bufs=2) as small,
            tc.tile_pool(name="nrmp", bufs=2) as nrmp,
            tc.tile_pool(name="pp", bufs=8, space="PSUM") as pp,
        ):
            # Tiles allocated up front; the DMAs that fill cpack / masks /
            # hT are emitted late (inside layer 0) so the DMA queue serves
            # layer-0's critical path (x80/xr0/wk) first.
            hT = pers.tile([P, DC * PT], F32R, tag="hT")
            hT3 = hT.rearrange("p (c t) -> p c t", c=DC)
            cpack = pers.tile([P, 132], F32, tag="cpack")
            nc.vector.memset(cpack[:, 0:1], 1.0)
            nc.vector.memset(cpack[0:1, 1:2], EPS)
            bcol = [cpack[:, 18 + 56 * l:18 + 56 * l + 56] for l in range(L)]
            eps_t = cpack[0:1, 1:2]
            ones_col = pers.tile([P, 1], F32R, tag="ones_col")
            nc.vector.tensor_copy(ones_col, cpack[:, 0:1])
            ones_bf = pers.tile([P, 1], BF16, tag="ones_bf")
            nc.vector.tensor_copy(ones_bf, cpack[:, 0:1])
            mk = pers.tile([P, NT * 2 * WW], BF16, tag="mk")
            ones_row = pers.tile([1, TC], BF16, tag="ones_row")
            nc.vector.memset(ones_row, 1.0)
            b2row = pers.tile([1, L * D], BF16, tag="b2row")

            def emit_late_dmas():
                nc.sync.dma_start(out=b2row, in_=b2r64)
                for l in range(L):
                    b0 = 18 + 56 * l
                    for i, v in enumerate((bq[l], bo[l], b2[l])):
                        nc.sync.dma_start(
                            out=cpack[:, b0 + 8 * i:b0 + 8 * i + 8],
                            in_=v.rearrange("(c p) -> p c", p=P))
                    nc.sync.dma_start(out=cpack[:, b0 + 24:b0 + 56],
                                      in_=b1[l].rearrange("(c p) -> p c", p=P))
                nc.sync.dma_start(out=mk, in_=masks_d)
                baseT3 = baseT.rearrange("p (c t) -> p c t", c=DC)
                for ci in range(NCH):
                    t0 = ci * TC
                    nc.sync.dma_start(out=hT3[:, :, t0:t0 + TC],
                                      in_=baseT3[:, :, t0:t0 + TC])

            def ln_chunk(ci, out_pair=None, gb=None):
                """LayerNorm over features (partitions) for token chunk ci.
                gb=(g,b): write (x-mu)*rstd*g+b back into hT (float32).
                out_pair=(x8,xr): write fp8 value + fp8 residual."""
                t0 = ci * TC
                ps1 = pp.tile([1, TC], F32, tag="mm", name=f"lns1_{ci}")
                ps2 = pp.tile([1, TC], F32, tag="mm", name=f"lns2_{ci}")
                for dc in range(DC):
                    hsl = hT[:, dc * PT + t0:dc * PT + t0 + TC]
                    nc.tensor.matmul(ps1, lhsT=ones_col, rhs=hsl,
                                     start=(dc == 0), stop=(dc == DC - 1))
                    sq = lnp.tile([P, TC], BF16, tag="lnsq", name=f"lnsq{dc}")
                    nc.scalar.square(sq, hsl)
                    nc.tensor.matmul(ps2, lhsT=ones_bf, rhs=sq,
                                     start=(dc == 0), stop=(dc == DC - 1))
                mean = small.tile([1, TC], F32, tag="smf", name="mean")
                var = small.tile([1, TC], F32, tag="smf", name="var")
                rstd = small.tile([1, TC], BF16, tag="smb", name="rstd")
                mr = small.tile([1, TC], BF16, tag="smb", name="mr")
                nc.vector.tensor_scalar_mul(mean, ps1, 1.0 / D)
                nc.vector.tensor_mul(var, mean, mean)
                nc.vector.scalar_tensor_tensor(var, ps2, 1.0 / D, var,
                                               op0=OP.mult, op1=OP.subtract)
                nc.scalar.activation(var, var, AF.Sqrt, bias=eps_t)
                nc.vector.reciprocal(rstd, var)
                nc.vector.tensor_mul(mr, mean, rstd)
                RM = rmp.tile([P, 2 * TC], BF16, tag="lnRM")
                nc.gpsimd.partition_broadcast(RM[:, 0:TC], rstd)
                nc.gpsimd.partition_broadcast(RM[:, TC:2 * TC], mr)
                for dc in range(DC):
                    hsl = hT[:, dc * PT + t0:dc * PT + t0 + TC]
                    d1 = lnp.tile([P, TC], F32, tag="lnt", name=f"lnd{dc}")
                    nc.vector.tensor_mul(d1, hsl, RM[:, 0:TC])
                    if gb is not None:
                        d2 = lnp.tile([P, TC], F32, tag="lnt", name=f"lnd2_{dc}")
                        nc.vector.tensor_sub(d2, d1, RM[:, TC:2 * TC])
                        gcol, bc = gb
                        nc.vector.tensor_scalar(
                            hsl, d2, gcol[:, dc:dc + 1], bc[:, dc:dc + 1],
                            op0=OP.mult, op1=OP.add)
                    else:
                        t = lnp.tile([P, TC], F32, tag="lnt", name=f"lnx{dc}")
                        nc.vector.tensor_sub(t, d1, RM[:, TC:2 * TC])
                        x8sl = out_pair[0][:, dc * PT + t0:dc * PT + t0 + TC]
                        r8sl = out_pair[1][:, dc * PT + t0:dc * PT + t0 + TC]
                        nc.scalar.activation(x8sl, t, AF.Copy)
                        nc.vector.tensor_sub(r8sl, t, x8sl)



            def dr_accum(ps, w8v, wrv, xa3, xr3a, c0, cw, t0, tw):
                """12 DoubleRow MMs: psum += (W8+R8)^T(x8+r8) minus R8r8,
                contraction over all 8 dc blocks (4 pairs x 3 terms)."""
                n = 0
                for dcp in range(4):
                    hf, lc = dcp // 2, (2 * dcp) % 4
                    w8sl = w8v[hf][:, lc:lc + 2, c0:c0 + cw]
                    wrsl = wrv[hf][:, lc:lc + 2, c0:c0 + cw]
                    x8sl = xa3[:, 2 * dcp:2 * dcp + 2, t0:t0 + tw]
                    xrsl = xr3a[:, 2 * dcp:2 * dcp + 2, t0:t0 + tw]
                    for lh, rh in ((w8sl, x8sl), (w8sl, xrsl), (wrsl, x8sl)):
                        nc.tensor.matmul(ps, lhsT=lh, rhs=rh,
                                         start=(n == 0), stop=(n == 11),
                                         perf_mode=DR)
                        n += 1

            def dr_accum_tok(ps, w8v, wrv, xa3, xr3a, c0, cw, tt):
                """Token-major variant: lhsT = activation pairs."""
                n = 0
                for dcp in range(4):
                    hf, lc = dcp // 2, (2 * dcp) % 4
                    x8sl = xa3[:, 2 * dcp:2 * dcp + 2, tt * 128:tt * 128 + 128]
                    xrsl = xr3a[:, 2 * dcp:2 * dcp + 2, tt * 128:tt * 128 + 128]
                    w8sl = w8v[hf][:, lc:lc + 2, c0:c0 + cw]
                    wrsl = wrv[hf][:, lc:lc + 2, c0:c0 + cw]
                    for lh, rh in ((x8sl, w8sl), (xrsl, w8sl), (x8sl, wrsl)):
                        nc.tensor.matmul(ps, lhsT=lh, rhs=rh,
                                         start=(n == 0), stop=(n == 11),
                                         perf_mode=DR)
                        n += 1

            def load_w8(dram8, dramr, tag, l):
                """Load an fp8 [D,D]-class matrix + residual as 2 half tiles
                each, returning ([P,4,1024] views x 2 halves) for both."""
                v8, vr = [], []
                for hf in range(2):
                    t8 = wp8.tile([P, 4 * D], FP8, tag="w8", name=f"{tag}8_{l}_{hf}")
                    nc.sync.dma_start(out=t8, in_=dram8[:, hf * 4 * D:(hf + 1) * 4 * D])
                    v8.append(t8.rearrange("p (c d) -> p c d", c=4))
                    tr = wp8.tile([P, 4 * D], FP8, tag="w8", name=f"{tag}r_{l}_{hf}")
                    nc.sync.dma_start(out=tr, in_=dramr[:, hf * 4 * D:(hf + 1) * 4 * D])
                    vr.append(tr.rearrange("p (c d) -> p c d", c=4))
                return v8, vr

            # ---------- layers ----------
            for l in range(L):
                # ---- LN1 -> x8 + residual (fp8, full shard) ----
                # layer 0's LN1 is input-only: host ships it pre-computed
                x8 = x8p.tile([P, DC * PT], FP8, tag="x8", name=f"x8_{l}")
                xr = x8p.tile([P, DC * PT], FP8, tag="xr", name=f"xr_{l}")
                wkb8 = wkbr = None
                if l == 0:
                    # weight DMAs first: K(c0) needs wk + chunk 0 only
                    wkb8, wkbr = load_w8(wk8[l], wkr[l], "wk", l)
                    x8c = x8.rearrange("p (c t) -> p c t", c=DC)
                    xrc = xr.rearrange("p (c t) -> p c t", c=DC)
                    x80c = x80_d.rearrange("p (c t) -> p c t", c=DC)
                    xr0c = xr0_d.rearrange("p (c t) -> p c t", c=DC)
                    for ci in range(NCH):
                        t0 = ci * TC
                        nc.sync.dma_start(out=x8c[:, :, t0:t0 + TC],
                                          in_=x80c[:, :, t0:t0 + TC])
                        nc.sync.dma_start(out=xrc[:, :, t0:t0 + TC],
                                          in_=xr0c[:, :, t0:t0 + TC])
                else:
                    for ci in range(NCH):
                        ln_chunk(ci, out_pair=(x8, xr))
                x83 = x8.rearrange("p (c t) -> p c t", c=DC)
                xr3 = xr.rearrange("p (c t) -> p c t", c=DC)

                # ---- K (feature-major KT) and V (token-major Vext) ----
                KT = kup.tile([P, DC * PT], BF16, tag="ku", name=f"KT{l}")
                Vx = vxp.tile([P, 65, NT * H], BF16, tag="vx", name=f"Vx{l}")
                nc.vector.memset(Vx[:, 64:65, :], 1.0)

                if wkb8 is None:
                    wkb8, wkbr = load_w8(wk8[l], wkr[l], "wk", l)
                for ci in range(NCH):
                    t0 = ci * TC
                    for og in range(2):
                        pss = [pp.tile([P, TC], F32, tag="mm", name=f"psk{i}")
                               for i in range(4)]
                        for o4 in range(4):
                            oc = og * 4 + o4
                            dr_accum(pss[o4], wkb8, wkbr, x83, xr3,
                                     oc * 128, 128, t0, TC)
                        for o4 in range(4):
                            oc = og * 4 + o4
                            nc.scalar.activation(
                                KT[:, oc * PT + t0:oc * PT + t0 + TC], pss[o4],
                                AF.Copy, scale=IWS)

                wvb8, wvbr = load_w8(wv8[l], wvr[l], "wv", l)
                for tt in range(NT):
                    for nh in range(2):
                        psv = pp.tile([P, 512], F32, tag="mm", name=f"psv{tt}_{nh}")
                        dr_accum_tok(psv, wvb8, wvbr, x83, xr3,
                                     nh * 512, 512, tt)
                        nc.scalar.activation(
                            Vx[:, 0:64, tt * H + nh * 8:tt * H + nh * 8 + 8],
                            psv.rearrange("p (h x) -> p x h", h=8),
                            AF.Copy, scale=IWS)

                # ---- Q (feature-major QT, bias bq') ----
                QT = qtp.tile([P, DC * PT], BF16, tag="qt", name=f"QT{l}")
                wqb8, wqbr = load_w8(wq8[l], wqr[l], "wq", l)
                if l == 0:
                    emit_late_dmas()
                for ci in range(NCH):
                    t0 = ci * TC
                    for og in range(2):
                        psq = [pp.tile([P, TC], F32, tag="mm", name=f"psq{i}")
                               for i in range(4)]
                        for o4 in range(4):
                            oc = og * 4 + o4
                            dr_accum(psq[o4], wqb8, wqbr, x83, xr3,
                                     oc * 128, 128, t0, TC)
                        for o4 in range(4):
                            oc = og * 4 + o4
                            nc.scalar.activation(
                                QT[:, oc * PT + t0:oc * PT + t0 + TC], psq[o4],
                                AF.Identity, bias=bcol[l][:, oc:oc + 1], scale=IWS)

                # ---- attention; ctx overwrites QT regions per head ----
                ests = {}

                def w0_of(j):
                    return min(max(j * 128 - AW, 0), PT - WW)

                def attn_scores(i):
                    est = estp.tile([P, NT * 384], BF16, tag="est", name=f"est{i}")
                    ests[i] = est
                    for j in range(NT):
                        w0 = w0_of(j)
                        for ho in range(2):
                            po = ho * 64
                            pst = pp.tile([P, 384], F32, tag="mm", name=f"pst{j}_{ho}")
                            nc.tensor.matmul(
                                pst[:, 0:WW],
                                lhsT=KT[po:po + 64, i * PT + j * 128:i * PT + j * 128 + 128],
                                rhs=QT[po:po + 64, i * PT + w0:i * PT + w0 + WW],
                                start=True, stop=True)
                            esl = est[:, j * 384 + ho * WW:j * 384 + (ho + 1) * WW]
                            nc.scalar.activation(esl, pst[:, 0:WW], AF.Exp,
                                                 scale=float(SCALE))
                        ej = est[:, j * 384:j * 384 + 2 * WW]
                        nc.vector.tensor_mul(ej, ej, mk[:, j * 2 * WW:(j + 1) * 2 * WW])

                def attn_ctx(i):
                    est = ests.pop(i)
                    for ho in range(2):
                        h = 2 * i + ho
                        po = ho * 64
                        for c in range(NCH):
                            psc = pp.tile([P, TC], F32, tag="mm", name=f"psc{h}_{c}")
                            mms = []
                            for qi in range(3):
                                qt = 3 * c + qi
                                w0c = w0_of(qt)
                                mms.append((qi * 128, 128, qt, qt * 128 - w0c, True))
                                if qt - 1 >= 0:
                                    w0e = w0_of(qt - 1)
                                    mms.append((qi * 128, AW, qt - 1,
                                                qt * 128 - w0e, False))
                                if qt + 1 < NT:
                                    w0e = w0_of(qt + 1)
                                    mms.append((qi * 128 + 128 - AW, AW, qt + 1,
                                                (qt + 1) * 128 - AW - w0e, False))
                            for mi, (pcol, wdt, j, ecol, st) in enumerate(mms):
                                nc.tensor.matmul(
                                    psc[0:65, pcol:pcol + wdt],
                                    lhsT=Vx[:, 0:65, j * H + h:j * H + h + 1],
                                    rhs=est[:, j * 384 + ho * WW + ecol:
                                            j * 384 + ho * WW + ecol + wdt],
                                    start=st, stop=(mi == len(mms) - 1),
                                    skip_group_check=True)
                            den = nrmp.tile([1, TC], BF16, tag="den", name=f"den{h}_{c}")
                            rnm = nrmp.tile([64, TC], BF16, tag="rnm", name=f"rnm{h}_{c}")
                            nc.vector.reciprocal(den, psc[64:65, :])
                            nc.gpsimd.partition_broadcast(rnm, den)
                            nc.vector.tensor_mul(
                                QT[po:po + 64, i * PT + c * TC:i * PT + c * TC + TC],
                                psc[0:64, :], rnm)

                attn_scores(0)
                for i in range(1, H // 2):
                    attn_scores(i)
                    attn_ctx(i - 1)
                attn_ctx(H // 2 - 1)

                # ---- O-projection + residual (ctx lives in QT; bf16) ----
                wob = []
                for hf in range(2):
                    t = wpb.tile([P, 4 * D], BF16, tag="w", name=f"wob{l}_{hf}")
                    nc.sync.dma_start(out=t, in_=wo[l][:, hf * 4 * D:(hf + 1) * 4 * D])
                    wob.append(t)
                for ci in range(NCH):
                    t0 = ci * TC
                    for og in range(2):
                        pso = [pp.tile([P, TC], F32, tag="mm", name=f"pso{i}")
                               for i in range(4)]
                        for di in range(DC):
                            wb = wob[di // 4]
                            for o4 in range(4):
                                do_ = og * 4 + o4
                                nc.tensor.matmul(
                                    pso[o4],
                                    lhsT=wb[:, (di % 4) * D + do_ * 128:(di % 4) * D + do_ * 128 + 128],
                                    rhs=QT[:, di * PT + t0:di * PT + t0 + TC],
                                    start=(di == 0), stop=(di == DC - 1))
                        for o4 in range(4):
                            do_ = og * 4 + o4
                            hsl = hT[:, do_ * PT + t0:do_ * PT + t0 + TC]
                            nc.vector.scalar_tensor_tensor(
                                hsl, pso[o4], bcol[l][:, 8 + do_:8 + do_ + 1], hsl,
                                op0=OP.add, op1=OP.add)

                # ---- LN2 -> x8/xr (reuses the x8p slots) ----
                x28 = x8p.tile([P, DC * PT], FP8, tag="x8", name=f"x28_{l}")
                x2r = x8p.tile([P, DC * PT], FP8, tag="xr", name=f"x2r_{l}")
                for ci in range(NCH):
                    ln_chunk(ci, out_pair=(x28, x2r))
                x283 = x28.rearrange("p (c t) -> p c t", c=DC)
                x2r3 = x2r.rearrange("p (c t) -> p c t", c=DC)

                # ---- FFN: W1 and W2 both fp8-DR, compensated uT ----
                for ci in range(NCH):
                    t0 = ci * TC
                    # u8 in first half, its fp8 residual ur in second half
                    uT = kup.tile([P, 2 * FC * TC], FP8, tag="ku", name=f"uT{l}_{ci}")
                    UR = FC * TC
                    for fg in range(4):
                        p8, pr = [], []
                        for hf in range(2):
                            t8 = wp8.tile([P, 4096], FP8, tag="w8",
                                          name=f"w18p{l}_{ci}_{fg}_{hf}")
                            nc.sync.dma_start(
                                out=t8,
                                in_=w18[l][:, fg * 8192 + hf * 4096:fg * 8192 + (hf + 1) * 4096])
                            p8.append(t8.rearrange("p (c d) -> p c d", c=4))
                            tr = wp8.tile([P, 4096], FP8, tag="w8",
                                          name=f"w1rp{l}_{ci}_{fg}_{hf}")
                            nc.sync.dma_start(
                                out=tr,
                                in_=w1r[l][:, fg * 8192 + hf * 4096:fg * 8192 + (hf + 1) * 4096])
                            pr.append(tr.rearrange("p (c d) -> p c d", c=4))
                        for fgi in range(2):
                            psf = [pp.tile([P, TC], F32, tag="mm", name=f"psf{i}")
                                   for i in range(4)]
                            for f4 in range(4):
                                fcl = fgi * 4 + f4
                                dr_accum(psf[f4], p8, pr, x283, x2r3,
                                         fcl * 128, 128, t0, TC)
                            for f4 in range(4):
                                fc = fg * 8 + fgi * 4 + f4
                                gt = lnp.tile([P, TC], F32, tag="lnt",
                                              name=f"gt{fc}")
                                nc.scalar.activation(
                                    gt, psf[f4], AF.Gelu,
                                    bias=bcol[l][:, 24 + fc:24 + fc + 1], scale=IWS)
                                u8sl = uT[:, fc * TC:(fc + 1) * TC]
                                ursl = uT[:, UR + fc * TC:UR + (fc + 1) * TC]
                                if fc % 2 == 0:
                                    nc.scalar.activation(u8sl, gt, AF.Copy)
                                else:
                                    nc.vector.tensor_copy(u8sl, gt)
                                nc.vector.tensor_sub(ursl, gt, u8sl)
                    def w2_piece(psh, nmm, dg, pc):
                        wb8 = wp8.tile([P, 4096], FP8, tag="w8",
                                       name=f"w28p{l}_{ci}_{dg}_{pc}")
                        nc.sync.dma_start(
                            out=wb8,
                            in_=w2[l][0][:, dg * 16384 + pc * 4096:dg * 16384 + (pc + 1) * 4096])
                        wbr = wp8.tile([P, 4096], FP8, tag="w8",
                                       name=f"w2rp{l}_{ci}_{dg}_{pc}")
                        nc.sync.dma_start(
                            out=wbr,
                            in_=w2[l][1][:, dg * 16384 + pc * 4096:dg * 16384 + (pc + 1) * 4096])
                        w83 = wb8.rearrange("p (c d) -> p c d", c=8)
                        wr3 = wbr.rearrange("p (c d) -> p c d", c=8)
                        u83 = uT.rearrange("p (c t) -> p c t", c=2 * FC)
                        for fp_ in range(4):
                            fc = pc * 8 + 2 * fp_
                            u8sl = u83[:, fc:fc + 2, :]
                            ursl = u83[:, FC + fc:FC + fc + 2, :]
                            for o4 in range(4):
                                w8sl = w83[:, 2 * fp_:2 * fp_ + 2,
                                           o4 * 128:o4 * 128 + 128]
                                wrsl = wr3[:, 2 * fp_:2 * fp_ + 2,
                                           o4 * 128:o4 * 128 + 128]
                                for lh, rh in ((w8sl, u8sl), (w8sl, ursl),
                                               (wrsl, u8sl)):
                                    nc.tensor.matmul(
                                        psh[o4], lhsT=lh, rhs=rh,
                                        start=(nmm[o4] == 0), stop=False,
                                        perf_mode=DR)
                                    nmm[o4] += 1

                    for dg in range(2):
                        psh = [pp.tile([P, TC], F32, tag="mm", name=f"psh{i}")
                               for i in range(4)]
                        nmm = [0] * 4
                        for pc in range(4):
                            w2_piece(psh, nmm, dg, pc)
                        for o4 in range(4):
                            do_ = dg * 4 + o4
                            # + 64*b2 via a rank-1 matmul, closing the group
                            nc.tensor.matmul(
                                psh[o4],
                                lhsT=b2row[:, l * D + do_ * 128:l * D + do_ * 128 + 128],
                                rhs=ones_row,
                                start=False, stop=True)
                        for o4 in range(4):
                            do_ = dg * 4 + o4
                            hsl = hT[:, do_ * PT + t0:do_ * PT + t0 + TC]
                            nc.vector.scalar_tensor_tensor(
                                hsl, psh[o4], IWS, hsl,
                                op0=OP.mult, op1=OP.add)

            # chunked output DMA: chunk c leaves as soon as its last residual
            # add lands
            houtT3 = houtT.rearrange("p (c t) -> p c t", c=DC)
            for ci in range(NCH):
                t0 = ci * TC
                nc.sync.dma_start(out=houtT3[:, :, t0:t0 + TC],
                                  in_=hT3[:, :, t0:t0 + TC])

    nc.compile()
    return nc


_NC_CACHE = {}


def _get_nc():
    if "nc" not in _NC_CACHE:
        _NC_CACHE["nc"] = _build()
    return _NC_CACHE["nc"]


def _ln_np(x, g=None, bta=None):
    mu = x.mean(-1, keepdims=True)
    var = ((x - mu) ** 2).mean(-1, keepdims=True)
    y = (x - mu) / np.sqrt(var + EPS)
    if g is not None:
        y = y * g + bta
    return y.astype(np.float32)


def _prep_core(inputs, tokemb_f32, ln0g, ln0b, b, start, n):
    """Per-core in_map entries that depend on the shard."""
    ids = np.asarray(inputs["input_ids"][b, start:start + n]).astype(np.int64)
    pid = np.asarray(inputs["patch_ids"][b, start:start + n]).astype(np.int64)
    pos_emb = np.asarray(inputs["pos_emb"], np.float32)
    hashes = np.asarray(inputs["hash_embeddings"], np.float32)

    base = np.zeros((PT, D), np.float32)
    emb = tokemb_f32[ids] + pos_emb[start:start + n] + hashes[b, start:start + n]
    base[:n] = _ln_np(emb, ln0g, ln0b)
    baseT = np.ascontiguousarray(
        base.reshape(PT, DC, P).transpose(2, 1, 0).reshape(P, DC * PT))

    # layer-0 LN1 (gamma/beta folded into the weights) + fp8 split
    x0 = np.zeros((PT, D), np.float32)
    x0[:n] = _ln_np(base[:n])
    x0T = np.ascontiguousarray(
        x0.reshape(PT, DC, P).transpose(2, 1, 0).reshape(P, DC * PT))
    x80 = x0T.astype(E4)
    xr0 = (x0T - x80.astype(np.float32)).astype(E4)

    pidp = np.empty(PT, np.int64)
    pidp[:n] = pid
    pidp[n:] = -np.arange(1, PT - n + 1)

    runs = np.diff(np.concatenate(
        [[0], np.nonzero(np.diff(pidp))[0] + 1, [PT]]))
    if runs.max() > AW + 1:
        raise RuntimeError(
            f"patch run {runs.max()} exceeds attention window margin {AW + 1}")

    # per key tile j: WW-col query window [w0, w0+WW)
    m = np.zeros((NT, P, 2 * WW), np.float32)
    for j in range(NT):
        w0 = min(max(j * P - AW, 0), PT - WW)
        kk = pidp[j * P:(j + 1) * P]
        qq = pidp[w0:w0 + WW]
        blk = (kk[:, None] == qq[None, :]).astype(np.float32)
        m[j, :, 0:WW] = blk
        m[j, :, WW:2 * WW] = blk
    masks = np.ascontiguousarray(
        m.transpose(1, 0, 2).reshape(P, NT * 2 * WW)).astype(BF)
    return {"baseT": baseT, "x80": x80, "xr0": xr0, "masks": masks}


def _lay(w, nblk):
    """[nblk*128, C] -> [128, nblk*C] partition-major layout (no cast)."""
    C = w.shape[1]
    return np.ascontiguousarray(
        w.reshape(nblk, P, C).transpose(1, 0, 2).reshape(P, nblk * C))


def _fp8_pair(wlay):
    a = np.asarray(wlay * WS, np.float32)
    w8 = a.astype(E4)
    r8 = (a - w8.astype(np.float32)).astype(E4)
    return w8, r8


def kernel(**inputs):
    pid_all = np.asarray(inputs["patch_ids"])
    tokemb = np.asarray(inputs["tok_emb"], np.float32)

    ln0g = np.asarray(inputs["ln0_g"], np.float32)
    ln0b = np.asarray(inputs["ln0_b"], np.float32)
    shared = {}
    for l in range(L):
        g1 = np.asarray(inputs["ln1_g"][l], np.float32)
        n1 = np.asarray(inputs["ln1_b"][l], np.float32)
        g2 = np.asarray(inputs["ln2_g"][l], np.float32)
        n2 = np.asarray(inputs["ln2_b"][l], np.float32)
        Wq = np.asarray(inputs["Wq"][l], np.float32)
        Wk = np.asarray(inputs["Wk"][l], np.float32)
        Wv = np.asarray(inputs["Wv"][l], np.float32)
        Wo = np.asarray(inputs["Wo"][l], np.float32)
        W1 = np.asarray(inputs["W1"][l], np.float32)
        W2 = np.asarray(inputs["W2"][l], np.float32)

        bq_ = n1 @ Wq + np.asarray(inputs["bq"][l], np.float32)
        bv_eff = n1 @ Wv + np.asarray(inputs["bv"][l], np.float32)
        bo_ = bv_eff @ Wo + np.asarray(inputs["bo"][l], np.float32)
        b1_ = n2 @ W1 + np.asarray(inputs["b1"][l], np.float32)
        b2_ = np.asarray(inputs["b2"][l], np.float32)

        for nm, wmat, gg in (("wq", Wq, g1), ("wk", Wk, g1), ("wv", Wv, g1)):
            w8, r8 = _fp8_pair(_lay(gg[:, None] * wmat, DC))
            shared[f"{nm}8_{l}"] = w8
            shared[f"{nm}r_{l}"] = r8
        # W1 blocks ordered (fg, dc)
        w1lay = np.ascontiguousarray(
            (g2[:, None] * W1).reshape(DC, P, 4, 1024)
            .transpose(1, 2, 0, 3).reshape(P, DC * F))
        w8, r8 = _fp8_pair(w1lay)
        shared[f"w18_{l}"] = w8
        shared[f"w1r_{l}"] = r8
        shared[f"wo{l}"] = _lay(Wo, DC).astype(BF)
        # W2 blocks ordered (dg, fc): piece (dg,pc) = d-cols [dg*512,(dg+1)*512)
        w2lay = np.ascontiguousarray(
            W2.reshape(FC, P, 2, 512).transpose(1, 2, 0, 3).reshape(P, FC * D))
        w8, r8 = _fp8_pair(w2lay)
        shared[f"w28_{l}"] = w8
        shared[f"w2r_{l}"] = r8
        shared[f"bq{l}"] = np.ascontiguousarray(bq_)
        shared[f"bo{l}"] = np.ascontiguousarray(bo_)
        shared[f"b1{l}"] = np.ascontiguousarray(b1_)
        shared[f"b2{l}"] = np.ascontiguousarray(b2_)
    shared["b2r64"] = np.ascontiguousarray(
        (WS * np.stack([np.asarray(inputs["b2"][l], np.float32)
                        for l in range(L)]).reshape(1, L * D))).astype(BF)

    shards = []
    for b in range(B):
        pid = np.asarray(pid_all[b])
        bnd = np.nonzero(pid[1:] != pid[:-1])[0] + 1
        cand = bnd[(bnd >= S - PT) & (bnd <= PT)]
        if len(cand) == 0:
            raise RuntimeError("no patch boundary near S/2; cannot shard")
        s = int(cand[np.argmin(np.abs(cand - S // 2))])
        shards.append((b, 0, s))
        shards.append((b, s, S - s))

    in_maps = []
    for b, start, n in shards:
        m = dict(shared)
        m.update(_prep_core(inputs, tokemb, ln0g, ln0b, b, start, n))
        in_maps.append(m)

    nc = _get_nc()
    res = bass_utils.run_bass_kernel_spmd(nc, in_maps, core_ids=list(range(NCORES)))

    out = np.zeros((B, S, D), np.float32)
    for i, (b, start, n) in enumerate(shards):
        ht = res.results[i]["houtT"]
        hfull = ht.reshape(P, DC, PT).transpose(2, 1, 0).reshape(PT, D)
        out[b, start:start + n] = hfull[:n]
    return out


if __name__ == "__main__":
    _get_nc()
    print("built ok")
